# revision 1
# baseline (speedup 1.0000x reference)
"""Bond-centered tensor-moment descriptor kernel for Trainium2 (8 NeuronCores).

v2: edges sharded 8 ways; per-core pipeline per 4-eblock superblock:
  - bf16 atom table (f-major, b padded to 32); per-eblock indirect gather of
    both endpoints summed in-DMA (bf16)
  - rad fold via broadcast TT; Z[e,(fp,df,a,b26)] built with 9 tensor_scalar
    ops in full bf16 (DVE 4x mode); per-core hoisted geometry
  - PE transposes Z chunks into PSUM pairs, copies balanced over DVE/Act/Pool
  - one stationary matmul chain per f-pair; po copied to bf16 and DMAed by the
    SP queue into a transposed DRAM output [800, EPC]; host de-transposes
"""
import math
import numpy as np
import ml_dtypes

import concourse.bass as bass
from concourse import mybir
from concourse.bass import AP
from concourse.bass_utils import run_bass_kernel_spmd
from concourse.masks import make_identity
from concourse.tile import TileContext, ScopedClock

BF = ml_dtypes.bfloat16

# ----------------------------------------------------------------------------
# Problem constants
# ----------------------------------------------------------------------------
CUTOFF = 5.0
MAX_BASIS_DEG = 2
MAX_DEG = 4
N_ATOMS = 20000
N_EDGES = 50000
F = 16
N_CORES = 8

NSH = (MAX_BASIS_DEG + 1) ** 2        # 9
NB = (MAX_DEG + 1) ** 2               # 25
BPAD = 26                             # b pad in Z / W rows
B32 = 32                              # b pad in atom table (f-major rows)
NC_OUT = 2 * NB                       # 50
ABLK = NSH * BPAD                     # 234
FPBLK = 512                           # K rows per f-pair block (468 + 44 pad)
ZCOLS = 8 * FPBLK                     # 4096
NCHUNK = ZCOLS // 128                 # 32
EBLK = 128
NBLK = 49                             # ceil(6250 / 128)
EPC = NBLK * EBLK                     # 6272
SHARD = N_EDGES // N_CORES            # 6250
SUPER = 4

PATHS = [(l1, l2, l3)
         for l1 in range(MAX_BASIS_DEG + 1)
         for l2 in range(MAX_DEG + 1)
         for l3 in range(abs(l1 - l2), min(l1 + l2, MAX_DEG) + 1)]


# ----------------------------------------------------------------------------
# Clebsch-Gordan (host)
# ----------------------------------------------------------------------------
def _fac(n):
    return math.factorial(n)


def _cg(j1, m1, j2, m2, j3, m3):
    if m1 + m2 != m3:
        return 0.0
    if j3 < abs(j1 - j2) or j3 > j1 + j2:
        return 0.0
    pre = math.sqrt((2 * j3 + 1) * _fac(j3 + j1 - j2) * _fac(j3 - j1 + j2)
                    * _fac(j1 + j2 - j3) / _fac(j1 + j2 + j3 + 1))
    pre *= math.sqrt(_fac(j3 + m3) * _fac(j3 - m3) * _fac(j1 - m1) * _fac(j1 + m1)
                     * _fac(j2 - m2) * _fac(j2 + m2))
    s = 0.0
    for k in range(max(0, j2 - j3 - m1, j1 - j3 + m2),
                   min(j1 + j2 - j3, j1 - m1, j2 + m2) + 1):
        s += (-1) ** k / (_fac(k) * _fac(j1 + j2 - j3 - k) * _fac(j1 - m1 - k)
                          * _fac(j2 + m2 - k) * _fac(j3 - j2 + m1 + k)
                          * _fac(j3 - j1 - m2 + k))
    return pre * s


def _umat(l):
    U = np.zeros((2 * l + 1, 2 * l + 1), dtype=np.complex128)
    s2 = 1.0 / np.sqrt(2.0)
    for m in range(-l, l + 1):
        if m > 0:
            U[m + l, m + l] = ((-1) ** m) * s2
            U[m + l, -m + l] = s2
        elif m == 0:
            U[l, l] = 1.0
        else:
            am = -m
            U[m + l, m + l] = 1j * s2
            U[m + l, am + l] = -1j * ((-1) ** am) * s2
    return U


def _real_cg(l1, l2, l3):
    C = np.zeros((2 * l1 + 1, 2 * l2 + 1, 2 * l3 + 1), dtype=np.complex128)
    for m1 in range(-l1, l1 + 1):
        for m2 in range(-l2, l2 + 1):
            m3 = m1 + m2
            if -l3 <= m3 <= l3:
                C[m1 + l1, m2 + l2, m3 + l3] = _cg(l1, m1, l2, m2, l3, m3)
    G = np.einsum('aA,bB,cC,ABC->abc', _umat(l1), _umat(l2),
                  np.conj(_umat(l3)), C)
    G = G.real if (l1 + l2 + l3) % 2 == 0 else G.imag
    return np.ascontiguousarray(G)


def _build_weight_tensor(tp_weights):
    """[8*FPBLK, 100] stationary; row fp*512 + df*234 + a*26 + b, col 2c+df."""
    G_abc = np.zeros((NSH, NB, NC_OUT), dtype=np.float64)
    for p, (l1, l2, l3) in enumerate(PATHS):
        G = _real_cg(l1, l2, l3)
        par = (l1 + l2 + l3) % 2
        for ai in range(2 * l1 + 1):
            for bi in range(2 * l2 + 1):
                for ci in range(2 * l3 + 1):
                    v = G[ai, bi, ci]
                    if v != 0.0:
                        G_abc[l1 * l1 + ai, l2 * l2 + bi,
                              par * NB + l3 * l3 + ci] = v
    path_idx = {p: i for i, p in enumerate(PATHS)}
    l_of_a = [0, 1, 1, 1, 2, 2, 2, 2, 2]
    l_of_b = [int(np.sqrt(b)) for b in range(NB)]
    l_of_c = [int(np.sqrt(c % NB)) for c in range(NC_OUT)]

    W = np.zeros((F, NSH, BPAD, NC_OUT), dtype=np.float64)
    for ga in range(NSH):
        for gb in range(NB):
            for gc in np.nonzero(G_abc[ga, gb])[0]:
                p = path_idx[(l_of_a[ga], l_of_b[gb], l_of_c[gc])]
                for f in range(F):
                    W[f, ga, gb, gc] = G_abc[ga, gb, gc] * float(tp_weights[p, f])
    W = W.reshape(F, ABLK, NC_OUT)
    out = np.zeros((8, FPBLK, 2 * NC_OUT), dtype=np.float64)
    for fp in range(8):
        for df in range(2):
            out[fp, df * ABLK:(df + 1) * ABLK, df::2] = W[2 * fp + df]
    return out.reshape(8 * FPBLK, 2 * NC_OUT)


# ----------------------------------------------------------------------------
# Walrus single-sync-wait patches
# ----------------------------------------------------------------------------
def _drain_and_barrier_patched(self, tick_clock, wait_clock):
    nc = self.nc
    drain_inst = nc.sync.drain()
    wait_clock.add_sem_waits(drain_inst.ins,
                             ScopedClock({None: tick_clock.global_clock}))
    si = drain_inst.ins.sync_info
    waits = list(si.on_wait) if si else []
    if len(waits) > 1:
        drain_inst.ins.sync_info = mybir.SyncInfo(on_wait=[waits[0]],
                                                  on_update=list(si.on_update))
        for w in waits[1:]:
            d2 = nc.sync.drain()
            d2.ins.sync_info = mybir.SyncInfo(on_wait=[w], on_update=[])
    nc.all_engine_barrier()
    assert self.sems is not None
    popped = nc._tile_sem_poison_stack.pop()
    assert popped is self._sem_poison
    nc.clear_and_free_semaphores(list(self.sems.allocated().values()))
    nc.all_engine_barrier()


TileContext._drain_and_barrier = _drain_and_barrier_patched


def _split_multi_waits(nc):
    for f in nc.m.functions:
        for bb in f.blocks:
            newl = []
            changed = False
            for inst in bb.instructions:
                si = inst.sync_info
                waits = list(si.on_wait) if si else []
                if len(waits) > 1:
                    changed = True
                    for k, w in enumerate(waits[:-1]):
                        nop = mybir.InstDrain(name=f"{inst.name}-sw{k}",
                                              ins=[], outs=[])
                        nop.engine = inst.engine
                        nop.sync_info = mybir.SyncInfo(on_wait=[w], on_update=[])
                        newl.append(nop)
                    inst.sync_info = mybir.SyncInfo(on_wait=[waits[-1]],
                                                    on_update=list(si.on_update))
                newl.append(inst)
            if changed:
                bb.instructions = newl


# ----------------------------------------------------------------------------
# Device kernel
# ----------------------------------------------------------------------------
_NC_CACHE = {}

# engine assignment for the 16 psum->sbuf pair copies (chunk pairs 0..15)
# and the 8 po copies, tuned for balance (gpsimd cannot touch PSUM)
PAIR_ENGINE = (['v', 'a'] * 8)
PO_ENGINE = ['a'] * 8


def _build_bass(split_waits=True):
    nc = bass.Bass("TRN2", target_bir_lowering=False, debug=False)
    dt = mybir.dt
    f32 = dt.float32
    bf16 = dt.bfloat16

    a2 = nc.dram_tensor("a2", [N_ATOMS, F * B32], bf16, kind="ExternalInput").ap()
    idx = nc.dram_tensor("idx", [128, NBLK * 2], dt.int32, kind="ExternalInput").ap()
    disp = nc.dram_tensor("disp", [128, NBLK * 4], f32, kind="ExternalInput").ap()
    wmat = nc.dram_tensor("wmat", [128, 32 * 2 * NC_OUT], bf16,
                          kind="ExternalInput").ap()
    outT = nc.dram_tensor("outT", [8 * 2 * NC_OUT, EPC], bf16,
                          kind="ExternalOutput").ap()

    from contextlib import ExitStack
    with TileContext(nc) as tc, ExitStack() as ctx:
        consts = ctx.enter_context(tc.tile_pool(name="consts", bufs=1))
        wpool = ctx.enter_context(tc.tile_pool(name="wpool", bufs=1))
        geom = ctx.enter_context(tc.tile_pool(name="geom", bufs=1))
        zper = ctx.enter_context(tc.tile_pool(name="zper", bufs=1))   # Z + zts persistent
        gpool = ctx.enter_context(tc.tile_pool(name="gpool", bufs=3))  # gathered y
        ypool = ctx.enter_context(tc.tile_pool(name="ypool", bufs=3))  # rad-folded y
        opool = ctx.enter_context(tc.tile_pool(name="opool", bufs=3))  # po sbuf bf16
        pst = ctx.enter_context(tc.tile_pool(name="pst", bufs=5, space="PSUM"))
        psm = ctx.enter_context(tc.tile_pool(name="psm", bufs=3, space="PSUM"))

        # ---- constants ----
        identb = consts.tile([128, 128], bf16)
        make_identity(nc, identb[:])
        krow = consts.tile([128, F], f32)
        kint = consts.tile([128, F], dt.int32)
        nc.gpsimd.iota(kint[:], pattern=[[1, F]], base=1, channel_multiplier=0)
        nc.vector.tensor_copy(out=krow[:], in_=kint[:])
        biasC = consts.tile([128, 1], f32)
        nc.vector.memset(biasC[:], CUTOFF)

        # ---- hoisted geometry inputs first (keep SP queue clear) ----
        disp_t = geom.tile([128, NBLK, 4], f32)
        nc.sync.dma_start(out=disp_t[:], in_=disp[:, :])
        idx_t = geom.tile([128, NBLK, 2], dt.int32)
        nc.sync.dma_start(out=idx_t[:], in_=idx[:, :])

        # ---- stationary W pieces: one DMA, host pre-laid as [128, 32, 100] ----
        PIECES = [(0, 128), (128, 256), (256, 384), (384, 468)]
        wbig = wpool.tile([128, 32, 2 * NC_OUT], bf16, name="wbig")
        nc.scalar.dma_start(
            out=wbig[:].rearrange("p q m -> p (q m)"), in_=wmat[:, :])
        wt = {(fp, pi): (wbig[:, 4 * fp + pi, :] if pi < 3
                         else wbig[0:84, 4 * fp + pi, :])
              for fp in range(8) for pi in range(4)}

        NJ = NBLK  # 49
        sq = geom.tile([128, NJ, 3], f32)
        r2 = geom.tile([128, NJ], f32)
        r = geom.tile([128, NJ], f32)
        rm = geom.tile([128, NJ], f32)
        rinv = geom.tile([128, NJ], f32)
        u = geom.tile([128, NJ, 3], f32)
        msgn = geom.tile([128, NJ], f32)
        mask = geom.tile([128, NJ], f32)
        shf = geom.tile([128, NJ, NSH], f32)
        t6 = geom.tile([128, NJ], f32)
        t8 = geom.tile([128, NJ], f32)
        t8b = geom.tile([128, NJ], f32)
        rc = geom.tile([128, NJ], f32)
        x = geom.tile([128, NJ, F], f32)
        px = geom.tile([128, NJ, F], f32)
        prec = geom.tile([128, NJ, F], f32)
        th = geom.tile([128, NJ, F], f32)
        tf_ = geom.tile([128, NJ, F], f32)
        q = geom.tile([128, NJ, F], f32)
        sins = geom.tile([128, NJ, F], f32)
        radf = geom.tile([128, NJ, F], f32)
        radb = geom.tile([128, NJ, F], bf16)

        def emit_geometry(j0, j1):
            """Geometry chain for blocks [j0, j1) — sliced to overlap pipeline."""
            s = slice(j0, j1)
            nj = j1 - j0
            nc.scalar.square(sq[:, s, :], disp_t[:, s, 0:3])
            nc.vector.tensor_reduce(out=r2[:, s], in_=sq[:, s, :],
                                    op=mybir.AluOpType.add,
                                    axis=mybir.AxisListType.X)
            nc.scalar.sqrt(r[:, s], r2[:, s])
            nc.vector.tensor_scalar(out=rm[:, s], in0=r[:, s], scalar1=1e-9,
                                    scalar2=None, op0=mybir.AluOpType.max)
            nc.vector.reciprocal(rinv[:, s], rm[:, s])
            nc.vector.tensor_tensor(
                out=u[:, s, :], in0=disp_t[:, s, 0:3],
                in1=rinv[:, s, None].to_broadcast([128, nj, 3]),
                op=mybir.AluOpType.mult)
            nc.scalar.activation(msgn[:, s], r[:, s],
                                 mybir.ActivationFunctionType.Sign,
                                 bias=biasC[:, 0:1], scale=-1.0)
            nc.vector.tensor_scalar(out=mask[:, s], in0=msgn[:, s], scalar1=0.5,
                                    scalar2=0.5, op0=mybir.AluOpType.mult,
                                    op1=mybir.AluOpType.add)
            c1 = 0.4886025119029199
            c2 = 1.0925484305920792
            ux, uy, uz = u[:, s, 0:1], u[:, s, 1:2], u[:, s, 2:3]
            nc.vector.memset(shf[:, s, 0:1], 0.28209479177387814)
            nc.vector.tensor_scalar(out=shf[:, s, 1:2], in0=uy, scalar1=c1,
                                    scalar2=None, op0=mybir.AluOpType.mult)
            nc.vector.tensor_scalar(out=shf[:, s, 2:3], in0=uz, scalar1=c1,
                                    scalar2=None, op0=mybir.AluOpType.mult)
            nc.vector.tensor_scalar(out=shf[:, s, 3:4], in0=ux, scalar1=c1,
                                    scalar2=None, op0=mybir.AluOpType.mult)
            nc.vector.scalar_tensor_tensor(out=shf[:, s, 4:5], in0=ux, scalar=c2,
                                           in1=uy, op0=mybir.AluOpType.mult,
                                           op1=mybir.AluOpType.mult)
            nc.vector.scalar_tensor_tensor(out=shf[:, s, 5:6], in0=uy, scalar=c2,
                                           in1=uz, op0=mybir.AluOpType.mult,
                                           op1=mybir.AluOpType.mult)
            nc.vector.scalar_tensor_tensor(out=t6[:, s, None], in0=uz, scalar=3.0,
                                           in1=uz, op0=mybir.AluOpType.mult,
                                           op1=mybir.AluOpType.mult)
            nc.scalar.activation(shf[:, s, 6:7], t6[:, s, None],
                                 mybir.ActivationFunctionType.Copy,
                                 bias=-0.31539156525252005,
                                 scale=0.31539156525252005)
            nc.vector.scalar_tensor_tensor(out=shf[:, s, 7:8], in0=ux, scalar=c2,
                                           in1=uz, op0=mybir.AluOpType.mult,
                                           op1=mybir.AluOpType.mult)
            nc.vector.scalar_tensor_tensor(out=t8[:, s, None], in0=ux,
                                           scalar=0.5 * c2, in1=ux,
                                           op0=mybir.AluOpType.mult,
                                           op1=mybir.AluOpType.mult)
            nc.vector.scalar_tensor_tensor(out=t8b[:, s, None], in0=uy,
                                           scalar=-0.5 * c2, in1=uy,
                                           op0=mybir.AluOpType.mult,
                                           op1=mybir.AluOpType.mult)
            nc.vector.tensor_add(out=shf[:, s, 8:9], in0=t8[:, s, None],
                                 in1=t8b[:, s, None])
            # rad = sinc(k r / C) * mask  (bf16)
            nc.vector.tensor_scalar(out=rc[:, s], in0=rm[:, s],
                                    scalar1=1.0 / CUTOFF, scalar2=None,
                                    op0=mybir.AluOpType.mult)
            nc.vector.tensor_tensor(
                out=x[:, s, :],
                in0=rc[:, s, None].to_broadcast([128, nj, F]),
                in1=krow[:, None, :].to_broadcast([128, nj, F]),
                op=mybir.AluOpType.mult)
            nc.scalar.activation(px[:, s, :], x[:, s, :],
                                 mybir.ActivationFunctionType.Copy,
                                 bias=0.0, scale=math.pi)
            nc.vector.reciprocal(prec[:, s, :], px[:, s, :])
            MAGIC = 8388608.0
            nc.vector.tensor_scalar(out=th[:, s, :], in0=x[:, s, :], scalar1=0.5,
                                    scalar2=MAGIC, op0=mybir.AluOpType.mult,
                                    op1=mybir.AluOpType.add)
            nc.vector.tensor_scalar(out=tf_[:, s, :], in0=th[:, s, :],
                                    scalar1=-MAGIC, scalar2=None,
                                    op0=mybir.AluOpType.add)
            nc.vector.scalar_tensor_tensor(out=q[:, s, :], in0=tf_[:, s, :],
                                           scalar=-2.0, in1=x[:, s, :],
                                           op0=mybir.AluOpType.mult,
                                           op1=mybir.AluOpType.add)
            nc.scalar.activation(sins[:, s, :], q[:, s, :],
                                 mybir.ActivationFunctionType.Sin,
                                 bias=0.0, scale=math.pi)
            nc.vector.tensor_tensor(out=radf[:, s, :], in0=sins[:, s, :],
                                    in1=prec[:, s, :], op=mybir.AluOpType.mult)
            nc.vector.tensor_tensor(
                out=radb[:, s, :], in0=radf[:, s, :],
                in1=mask[:, s, None].to_broadcast([128, nj, F]),
                op=mybir.AluOpType.mult)

        # ---- persistent Z tiles (pad cols memset once) ----
        zs = [zper.tile([128, ZCOLS], bf16, name=f"z{i}") for i in range(SUPER)]
        for z in zs:
            zap = z[:]
            nc.gpsimd.memset(
                AP(zap.tensor, zap.offset + 468,
                   [list(zap.ap[0]), [FPBLK, 8], [1, FPBLK - 468]]), 0.0)
        # persistent Z^T pair tiles [128, 2, 512]
        zts = [zper.tile([128, 2, 512], bf16, name=f"zt{cp}")
               for cp in range(NCHUNK // 2)]

        def do_superblock(ebs):
            nebs = len(ebs)
            ne = nebs * 128
            yps = []
            for i, j in enumerate(ebs):
                g = gpool.tile([128, F * B32], bf16, tag=f"g{i}")
                nc.gpsimd.indirect_dma_start(
                    out=g[:], out_offset=None, in_=a2[:],
                    in_offset=bass.IndirectOffsetOnAxis(
                        ap=idx_t[:, j, 0:1], axis=0))
                nc.gpsimd.indirect_dma_start(
                    out=g[:], out_offset=None, in_=a2[:],
                    in_offset=bass.IndirectOffsetOnAxis(
                        ap=idx_t[:, j, 1:2], axis=0),
                    compute_op=mybir.AluOpType.add)
                yp = ypool.tile([128, F * B32], bf16, tag=f"yp{i}")
                # alternate Pool/DVE so radfolds don't serialize behind the
                # in-order Pool queue's gathers (and vice versa)
                eng = nc.gpsimd if i == 3 else nc.vector
                eng.tensor_tensor(
                    out=yp[:].rearrange("p (f b) -> p f b", f=F),
                    in0=g[:].rearrange("p (f b) -> p f b", f=F),
                    in1=radb[:, j, :, None].to_broadcast([128, F, B32]),
                    op=mybir.AluOpType.mult)
                yps.append(yp)

                # Z build: 9 tensor_scalar (bf16 4x) per eblock
                zap = zs[i][:]
                ypap = yp[:]
                for a in range(NSH):
                    zsl = AP(zap.tensor, zap.offset + a * BPAD,
                             [list(zap.ap[0]), [FPBLK, 8], [ABLK, 2], [1, BPAD]])
                    ysl = AP(ypap.tensor, ypap.offset,
                             [list(ypap.ap[0]), [2 * B32, 8], [B32, 2], [1, BPAD]])
                    nc.vector.tensor_scalar(out=zsl, in0=ysl,
                                            scalar1=shf[:, j, a:a + 1],
                                            scalar2=None,
                                            op0=mybir.AluOpType.mult)

            # transpose all chunks; copy pairs psum->sbuf
            for cp in range(NCHUNK // 2):
                pt = pst.tile([128, 2, 512], bf16, tag="pt", space="PSUM")
                for h in range(2):
                    c = 2 * cp + h
                    for i in range(nebs):
                        nc.tensor.transpose(
                            out=pt[:, h, i * 128:(i + 1) * 128],
                            in_=zs[i][:, c * 128:(c + 1) * 128],
                            identity=identb[:])
                eng = PAIR_ENGINE[cp]
                dst = zts[cp][:, :, :ne]
                src = pt[:, :, :ne]
                if eng == 'v':
                    nc.vector.tensor_copy(out=dst, in_=src)
                else:
                    nc.scalar.copy(out=dst, in_=src)

            # matmuls + po copy + out DMA
            e0 = ebs[0] * EBLK
            for fp in range(8):
                po = psm.tile([2 * NC_OUT, 512], f32, tag="po", space="PSUM")
                for pi, (r0, r1) in enumerate(
                        [(0, 128), (128, 256), (256, 384), (384, 468)]):
                    cp, h = divmod(4 * fp + pi, 2)
                    rhs = zts[cp][:, h, :ne] if r1 - r0 == 128 \
                        else zts[cp][0:84, h, :ne]
                    nc.tensor.matmul(out=po[:, :ne], lhsT=wt[(fp, pi)],
                                     rhs=rhs, start=(pi == 0), stop=(pi == 3))
                pos = opool.tile([2 * NC_OUT, 512], bf16, tag=f"pos{fp}")
                if PO_ENGINE[fp] == 'v':
                    nc.vector.tensor_copy(out=pos[:, :ne], in_=po[:, :ne])
                else:
                    nc.scalar.copy(out=pos[:, :ne], in_=po[:, :ne])
                nc.sync.dma_start(
                    out=outT[fp * 100:(fp + 1) * 100, e0:e0 + ne],
                    in_=pos[:, :ne])

        # geometry sliced: slice k covers blocks for superblocks 4k..4k+3,
        # emitted just before superblock 4(k-?) ... first slice up front,
        # later slices interleave so pipeline fill stays short
        # partial superblock (1 eblock) runs FIRST: it fills the pipeline
        # quickly and the kernel drains on a fully-pipelined superblock.
        # geometry sliced: tiny first slices, then 8-block slices emitted
        # ~2 superblocks ahead
        NSB = (NBLK - 1) // SUPER  # 12 full superblocks after the partial
        emit_geometry(NSB * SUPER, NBLK)     # block 48 only
        do_superblock([NSB * SUPER])
        emitted = 0
        for sb in range(NSB):
            if sb == 0:
                need = SUPER
            elif sb % 2 == 1:
                need = min((sb + 3) * SUPER, NSB * SUPER)
            else:
                need = emitted
            if need > emitted:
                emit_geometry(emitted, need)
                emitted = need
            do_superblock(list(range(sb * SUPER, (sb + 1) * SUPER)))

    if split_waits:
        _split_multi_waits(nc)
    return nc


def _get_nc():
    if "nc" not in _NC_CACHE:
        _NC_CACHE["nc"] = _build_bass()
    return _NC_CACHE["nc"]


# ----------------------------------------------------------------------------
# Host entry point
# ----------------------------------------------------------------------------
def kernel(atomic_descriptors, tp_weights, neighbour_displacements,
           neighbour_indices):
    atomic_descriptors = np.asarray(atomic_descriptors, dtype=np.float32)
    tp_weights = np.asarray(tp_weights, dtype=np.float32)
    neighbour_displacements = np.asarray(neighbour_displacements, dtype=np.float32)
    neighbour_indices = np.asarray(neighbour_indices, dtype=np.int32)

    # atom table: (A, 1, 25, 16) -> (A, 16, 32) f-major bf16
    A = atomic_descriptors.reshape(N_ATOMS, NB, F)
    a2 = np.zeros((N_ATOMS, F, B32), dtype=BF)
    a2[:, :, :NB] = A.transpose(0, 2, 1).astype(BF)
    a2 = a2.reshape(N_ATOMS, F * B32)

    wm = _build_weight_tensor(tp_weights).astype(BF)      # [4096, 100]
    # device layout [128, 32*100]: wmat[p, q*100+m] = wm[q*128+p, m]
    wmat = np.ascontiguousarray(
        wm.reshape(32, 128, 2 * NC_OUT).transpose(1, 0, 2)).reshape(128, -1)

    in_maps = []
    for c in range(N_CORES):
        idx_full = np.zeros((EPC, 2), dtype=np.int32)
        disp_full = np.ones((EPC, 3), dtype=np.float32)
        idx_full[:SHARD] = neighbour_indices[c * SHARD:(c + 1) * SHARD]
        disp_full[:SHARD] = neighbour_displacements[c * SHARD:(c + 1) * SHARD]
        # relayout to [128, NBLK, *]: edge j*128+p -> [p, j]
        idx2 = np.ascontiguousarray(
            idx_full.reshape(NBLK, 128, 2).transpose(1, 0, 2)).reshape(128, -1)
        disp4 = np.zeros((NBLK, 128, 4), dtype=np.float32)
        disp4[:, :, :3] = disp_full.reshape(NBLK, 128, 3)
        disp2 = np.ascontiguousarray(disp4.transpose(1, 0, 2)).reshape(128, -1)
        in_maps.append({"a2": a2, "idx": idx2, "disp": disp2, "wmat": wmat})

    nc = _get_nc()
    res = run_bass_kernel_spmd(nc, in_maps, core_ids=list(range(N_CORES)))

    out = np.empty((N_EDGES, 2, NB, F), dtype=np.float32)
    for c in range(N_CORES):
        oT = np.asarray(res.results[c]["outT"]).astype(np.float32)  # [800, EPC]
        # row fp*100 + 2*cc + df -> (f=2fp+df, par=cc//25, cm=cc%25)
        o = oT[:, :SHARD].reshape(8, 50, 2, SHARD)     # [fp, cc, df, e]
        o = o.transpose(3, 1, 0, 2).reshape(SHARD, 50, 16)  # [e, cc, f]
        o = o.reshape(SHARD, 2, 25, 16)
        out[c * SHARD:(c + 1) * SHARD] = o
    return out


if __name__ == "__main__":
    rng = np.random.default_rng(0)
    inputs = {
        "atomic_descriptors": rng.standard_normal(
            (N_ATOMS, 1, NB, F)).astype(np.float32),
        "tp_weights": (rng.standard_normal((len(PATHS), F)) * 0.1).astype(np.float32),
        "neighbour_displacements": (rng.standard_normal(
            (N_EDGES, 3)) * 1.5).astype(np.float32),
        "neighbour_indices": rng.integers(0, N_ATOMS, (N_EDGES, 2)).astype(np.int32),
    }
    out = kernel(**inputs)
    print("kernel ran, out shape", out.shape)



# revision 6
# speedup vs baseline: 1.0036x; 1.0036x over previous
"""Bond-centered tensor-moment descriptor kernel for Trainium2 (8 NeuronCores).

v2: edges sharded 8 ways; per-core pipeline per 4-eblock superblock:
  - bf16 atom table (f-major, b padded to 32); per-eblock indirect gather of
    both endpoints summed in-DMA (bf16)
  - rad fold via broadcast TT; Z[e,(fp,df,a,b26)] built with 9 tensor_scalar
    ops in full bf16 (DVE 4x mode); per-core hoisted geometry
  - PE transposes Z chunks into PSUM pairs, copies balanced over DVE/Act/Pool
  - one stationary matmul chain per f-pair; po copied to bf16 and DMAed by the
    SP queue into a transposed DRAM output [800, EPC]; host de-transposes
"""
import math
import numpy as np
import ml_dtypes

import concourse.bass as bass
from concourse import mybir
from concourse.bass import AP
from concourse.bass_utils import run_bass_kernel_spmd
from concourse.masks import make_identity
from concourse.tile import TileContext, ScopedClock

BF = ml_dtypes.bfloat16

# ----------------------------------------------------------------------------
# Problem constants
# ----------------------------------------------------------------------------
CUTOFF = 5.0
MAX_BASIS_DEG = 2
MAX_DEG = 4
N_ATOMS = 20000
N_EDGES = 50000
F = 16
N_CORES = 8

NSH = (MAX_BASIS_DEG + 1) ** 2        # 9
NB = (MAX_DEG + 1) ** 2               # 25
BPAD = 26                             # b pad in Z / W rows
B32 = 32                              # b pad in atom table (f-major rows)
NC_OUT = 2 * NB                       # 50
ABLK = NSH * BPAD                     # 234
FPBLK = 512                           # K rows per f-pair block (468 + 44 pad)
ZCOLS = 8 * FPBLK                     # 4096
NCHUNK = ZCOLS // 128                 # 32
EBLK = 128
NBLK = 49                             # ceil(6250 / 128)
EPC = NBLK * EBLK                     # 6272
SHARD = N_EDGES // N_CORES            # 6250
SUPER = 4

PATHS = [(l1, l2, l3)
         for l1 in range(MAX_BASIS_DEG + 1)
         for l2 in range(MAX_DEG + 1)
         for l3 in range(abs(l1 - l2), min(l1 + l2, MAX_DEG) + 1)]


# ----------------------------------------------------------------------------
# Clebsch-Gordan (host)
# ----------------------------------------------------------------------------
def _fac(n):
    return math.factorial(n)


def _cg(j1, m1, j2, m2, j3, m3):
    if m1 + m2 != m3:
        return 0.0
    if j3 < abs(j1 - j2) or j3 > j1 + j2:
        return 0.0
    pre = math.sqrt((2 * j3 + 1) * _fac(j3 + j1 - j2) * _fac(j3 - j1 + j2)
                    * _fac(j1 + j2 - j3) / _fac(j1 + j2 + j3 + 1))
    pre *= math.sqrt(_fac(j3 + m3) * _fac(j3 - m3) * _fac(j1 - m1) * _fac(j1 + m1)
                     * _fac(j2 - m2) * _fac(j2 + m2))
    s = 0.0
    for k in range(max(0, j2 - j3 - m1, j1 - j3 + m2),
                   min(j1 + j2 - j3, j1 - m1, j2 + m2) + 1):
        s += (-1) ** k / (_fac(k) * _fac(j1 + j2 - j3 - k) * _fac(j1 - m1 - k)
                          * _fac(j2 + m2 - k) * _fac(j3 - j2 + m1 + k)
                          * _fac(j3 - j1 - m2 + k))
    return pre * s


def _umat(l):
    U = np.zeros((2 * l + 1, 2 * l + 1), dtype=np.complex128)
    s2 = 1.0 / np.sqrt(2.0)
    for m in range(-l, l + 1):
        if m > 0:
            U[m + l, m + l] = ((-1) ** m) * s2
            U[m + l, -m + l] = s2
        elif m == 0:
            U[l, l] = 1.0
        else:
            am = -m
            U[m + l, m + l] = 1j * s2
            U[m + l, am + l] = -1j * ((-1) ** am) * s2
    return U


def _real_cg(l1, l2, l3):
    C = np.zeros((2 * l1 + 1, 2 * l2 + 1, 2 * l3 + 1), dtype=np.complex128)
    for m1 in range(-l1, l1 + 1):
        for m2 in range(-l2, l2 + 1):
            m3 = m1 + m2
            if -l3 <= m3 <= l3:
                C[m1 + l1, m2 + l2, m3 + l3] = _cg(l1, m1, l2, m2, l3, m3)
    G = np.einsum('aA,bB,cC,ABC->abc', _umat(l1), _umat(l2),
                  np.conj(_umat(l3)), C)
    G = G.real if (l1 + l2 + l3) % 2 == 0 else G.imag
    return np.ascontiguousarray(G)


def _build_weight_tensor(tp_weights):
    """[8*FPBLK, 100] stationary; row fp*512 + df*234 + a*26 + b, col 2c+df."""
    G_abc = np.zeros((NSH, NB, NC_OUT), dtype=np.float64)
    for p, (l1, l2, l3) in enumerate(PATHS):
        G = _real_cg(l1, l2, l3)
        par = (l1 + l2 + l3) % 2
        for ai in range(2 * l1 + 1):
            for bi in range(2 * l2 + 1):
                for ci in range(2 * l3 + 1):
                    v = G[ai, bi, ci]
                    if v != 0.0:
                        G_abc[l1 * l1 + ai, l2 * l2 + bi,
                              par * NB + l3 * l3 + ci] = v
    path_idx = {p: i for i, p in enumerate(PATHS)}
    l_of_a = [0, 1, 1, 1, 2, 2, 2, 2, 2]
    l_of_b = [int(np.sqrt(b)) for b in range(NB)]
    l_of_c = [int(np.sqrt(c % NB)) for c in range(NC_OUT)]

    W = np.zeros((F, NSH, BPAD, NC_OUT), dtype=np.float64)
    for ga in range(NSH):
        for gb in range(NB):
            for gc in np.nonzero(G_abc[ga, gb])[0]:
                p = path_idx[(l_of_a[ga], l_of_b[gb], l_of_c[gc])]
                for f in range(F):
                    W[f, ga, gb, gc] = G_abc[ga, gb, gc] * float(tp_weights[p, f])
    W = W.reshape(F, ABLK, NC_OUT)
    out = np.zeros((8, FPBLK, 2 * NC_OUT), dtype=np.float64)
    for fp in range(8):
        for df in range(2):
            out[fp, df * ABLK:(df + 1) * ABLK, df::2] = W[2 * fp + df]
    return out.reshape(8 * FPBLK, 2 * NC_OUT) / math.pi


# ----------------------------------------------------------------------------
# Walrus single-sync-wait patches
# ----------------------------------------------------------------------------
def _drain_and_barrier_patched(self, tick_clock, wait_clock):
    nc = self.nc
    drain_inst = nc.sync.drain()
    wait_clock.add_sem_waits(drain_inst.ins,
                             ScopedClock({None: tick_clock.global_clock}))
    si = drain_inst.ins.sync_info
    waits = list(si.on_wait) if si else []
    if len(waits) > 1:
        drain_inst.ins.sync_info = mybir.SyncInfo(on_wait=[waits[0]],
                                                  on_update=list(si.on_update))
        for w in waits[1:]:
            d2 = nc.sync.drain()
            d2.ins.sync_info = mybir.SyncInfo(on_wait=[w], on_update=[])
    nc.all_engine_barrier()
    assert self.sems is not None
    popped = nc._tile_sem_poison_stack.pop()
    assert popped is self._sem_poison
    nc.clear_and_free_semaphores(list(self.sems.allocated().values()))
    nc.all_engine_barrier()


TileContext._drain_and_barrier = _drain_and_barrier_patched


def _split_multi_waits(nc):
    for f in nc.m.functions:
        for bb in f.blocks:
            newl = []
            changed = False
            for inst in bb.instructions:
                si = inst.sync_info
                waits = list(si.on_wait) if si else []
                if len(waits) > 1:
                    changed = True
                    for k, w in enumerate(waits[:-1]):
                        nop = mybir.InstDrain(name=f"{inst.name}-sw{k}",
                                              ins=[], outs=[])
                        nop.engine = inst.engine
                        nop.sync_info = mybir.SyncInfo(on_wait=[w], on_update=[])
                        newl.append(nop)
                    inst.sync_info = mybir.SyncInfo(on_wait=[waits[-1]],
                                                    on_update=list(si.on_update))
                newl.append(inst)
            if changed:
                bb.instructions = newl


# ----------------------------------------------------------------------------
# Device kernel
# ----------------------------------------------------------------------------
_NC_CACHE = {}

# engine assignment for the 16 psum->sbuf pair copies (chunk pairs 0..15)
# and the 8 po copies, tuned for balance (gpsimd cannot touch PSUM)
PAIR_ENGINE = (['v', 'a'] * 8)
# Z-build engine per (eblock-in-superblock, a): 'v' DVE TSP, 'a' Act
# scale-AP activation, 'g' Pool TSP
Z_ENGINE = [['v'] * 9 for _ in range(4)]
PO_ENGINE = ['a'] * 8


def _build_bass(split_waits=True):
    nc = bass.Bass("TRN2", target_bir_lowering=False, debug=False)
    dt = mybir.dt
    f32 = dt.float32
    bf16 = dt.bfloat16

    a2 = nc.dram_tensor("a2", [N_ATOMS, F * B32], bf16, kind="ExternalInput").ap()
    idx = nc.dram_tensor("idx", [128, NBLK * 2], dt.int32, kind="ExternalInput").ap()
    disp = nc.dram_tensor("disp", [128, NBLK * 4], f32, kind="ExternalInput").ap()
    wmat = nc.dram_tensor("wmat", [128, 32 * 2 * NC_OUT], bf16,
                          kind="ExternalInput").ap()
    outT = nc.dram_tensor("outT", [8 * 2 * NC_OUT, EPC], bf16,
                          kind="ExternalOutput").ap()

    from contextlib import ExitStack
    with TileContext(nc) as tc, ExitStack() as ctx:
        consts = ctx.enter_context(tc.tile_pool(name="consts", bufs=1))
        wpool = ctx.enter_context(tc.tile_pool(name="wpool", bufs=1))
        geom = ctx.enter_context(tc.tile_pool(name="geom", bufs=1))
        zper = ctx.enter_context(tc.tile_pool(name="zper", bufs=1))   # Z + zts persistent
        gpool = ctx.enter_context(tc.tile_pool(name="gpool", bufs=3))  # gathered y
        ypool = ctx.enter_context(tc.tile_pool(name="ypool", bufs=3))  # rad-folded y
        opool = ctx.enter_context(tc.tile_pool(name="opool", bufs=3))  # po sbuf bf16
        pst = ctx.enter_context(tc.tile_pool(name="pst", bufs=5, space="PSUM"))
        psm = ctx.enter_context(tc.tile_pool(name="psm", bufs=3, space="PSUM"))

        # ---- constants ----
        identb = consts.tile([128, 128], bf16)
        make_identity(nc, identb[:])
        krow = consts.tile([128, F], f32)
        kint = consts.tile([128, F], dt.int32)
        nc.gpsimd.iota(kint[:], pattern=[[1, F]], base=1, channel_multiplier=0)
        nc.vector.tensor_copy(out=krow[:], in_=kint[:])

        # ---- hoisted geometry inputs first (keep SP queue clear) ----
        disp_t = geom.tile([128, NBLK, 4], f32)
        nc.sync.dma_start(out=disp_t[:], in_=disp[:, :])
        idx_t = geom.tile([128, NBLK, 2], dt.int32)
        nc.sync.dma_start(out=idx_t[:], in_=idx[:, :])

        # ---- stationary W pieces: one DMA, host pre-laid as [128, 32, 100] ----
        PIECES = [(0, 128), (128, 256), (256, 384), (384, 468)]
        wbig = wpool.tile([128, 32, 2 * NC_OUT], bf16, name="wbig")
        nc.scalar.dma_start(
            out=wbig[:].rearrange("p q m -> p (q m)"), in_=wmat[:, :])
        wt = {(fp, pi): (wbig[:, 4 * fp + pi, :] if pi < 3
                         else wbig[0:84, 4 * fp + pi, :])
              for fp in range(8) for pi in range(4)}

        NJ = NBLK  # 49
        sq = geom.tile([128, NJ, 3], f32)
        r2 = geom.tile([128, NJ], f32)
        r = geom.tile([128, NJ], f32)
        rm = geom.tile([128, NJ], f32)
        rinv = geom.tile([128, NJ], f32)
        u = geom.tile([128, NJ, 3], f32)
        shf = geom.tile([128, NJ, NSH], f32)
        t6 = geom.tile([128, NJ], f32)
        t8 = geom.tile([128, NJ], f32)
        t8b = geom.tile([128, NJ], f32)
        rc = geom.tile([128, NJ], f32)
        x = geom.tile([128, NJ, F], f32)
        prec = geom.tile([128, NJ, F], f32)
        th = geom.tile([128, NJ, F], f32)
        tf_ = geom.tile([128, NJ, F], f32)
        q = geom.tile([128, NJ, F], f32)
        sins = geom.tile([128, NJ, F], f32)
        radb = geom.tile([128, NJ, F], bf16)

        def emit_geometry(j0, j1):
            """Geometry chain for blocks [j0, j1) — sliced to overlap pipeline."""
            s = slice(j0, j1)
            nj = j1 - j0
            nc.scalar.square(sq[:, s, :], disp_t[:, s, 0:3])
            nc.vector.tensor_reduce(out=r2[:, s], in_=sq[:, s, :],
                                    op=mybir.AluOpType.add,
                                    axis=mybir.AxisListType.X)
            nc.scalar.sqrt(r[:, s], r2[:, s])
            nc.vector.tensor_scalar(out=rm[:, s], in0=r[:, s], scalar1=1e-9,
                                    scalar2=None, op0=mybir.AluOpType.max)
            nc.vector.reciprocal(rinv[:, s], rm[:, s])
            nc.vector.tensor_tensor(
                out=u[:, s, :], in0=disp_t[:, s, 0:3],
                in1=rinv[:, s, None].to_broadcast([128, nj, 3]),
                op=mybir.AluOpType.mult)
            c1 = 0.4886025119029199
            c2 = 1.0925484305920792
            ux, uy, uz = u[:, s, 0:1], u[:, s, 1:2], u[:, s, 2:3]
            nc.vector.memset(shf[:, s, 0:1], 0.28209479177387814)
            nc.vector.tensor_scalar(out=shf[:, s, 1:2], in0=uy, scalar1=c1,
                                    scalar2=None, op0=mybir.AluOpType.mult)
            nc.vector.tensor_scalar(out=shf[:, s, 2:3], in0=uz, scalar1=c1,
                                    scalar2=None, op0=mybir.AluOpType.mult)
            nc.vector.tensor_scalar(out=shf[:, s, 3:4], in0=ux, scalar1=c1,
                                    scalar2=None, op0=mybir.AluOpType.mult)
            nc.vector.scalar_tensor_tensor(out=shf[:, s, 4:5], in0=ux, scalar=c2,
                                           in1=uy, op0=mybir.AluOpType.mult,
                                           op1=mybir.AluOpType.mult)
            nc.vector.scalar_tensor_tensor(out=shf[:, s, 5:6], in0=uy, scalar=c2,
                                           in1=uz, op0=mybir.AluOpType.mult,
                                           op1=mybir.AluOpType.mult)
            nc.vector.scalar_tensor_tensor(out=t6[:, s, None], in0=uz, scalar=3.0,
                                           in1=uz, op0=mybir.AluOpType.mult,
                                           op1=mybir.AluOpType.mult)
            nc.scalar.activation(shf[:, s, 6:7], t6[:, s, None],
                                 mybir.ActivationFunctionType.Copy,
                                 bias=-0.31539156525252005,
                                 scale=0.31539156525252005)
            nc.vector.scalar_tensor_tensor(out=shf[:, s, 7:8], in0=ux, scalar=c2,
                                           in1=uz, op0=mybir.AluOpType.mult,
                                           op1=mybir.AluOpType.mult)
            nc.vector.scalar_tensor_tensor(out=t8[:, s, None], in0=ux,
                                           scalar=0.5 * c2, in1=ux,
                                           op0=mybir.AluOpType.mult,
                                           op1=mybir.AluOpType.mult)
            nc.vector.scalar_tensor_tensor(out=t8b[:, s, None], in0=uy,
                                           scalar=-0.5 * c2, in1=uy,
                                           op0=mybir.AluOpType.mult,
                                           op1=mybir.AluOpType.mult)
            nc.vector.tensor_add(out=shf[:, s, 8:9], in0=t8[:, s, None],
                                 in1=t8b[:, s, None])
            # rad = sinc(k r / C) * mask  (bf16)
            # rc = min(r/C, 1): sinc(k*1) == 0 for integer k, so the
            # cutoff mask is implicit in the clamp
            nc.vector.tensor_scalar(out=rc[:, s], in0=rm[:, s],
                                    scalar1=1.0 / CUTOFF, scalar2=1.0,
                                    op0=mybir.AluOpType.mult,
                                    op1=mybir.AluOpType.min)
            nc.vector.tensor_tensor(
                out=x[:, s, :],
                in0=rc[:, s, None].to_broadcast([128, nj, F]),
                in1=krow[:, None, :].to_broadcast([128, nj, F]),
                op=mybir.AluOpType.mult)
            nc.vector.reciprocal(prec[:, s, :], x[:, s, :])
            MAGIC = 8388608.0
            nc.vector.tensor_scalar(out=th[:, s, :], in0=x[:, s, :], scalar1=0.5,
                                    scalar2=MAGIC, op0=mybir.AluOpType.mult,
                                    op1=mybir.AluOpType.add)
            nc.vector.tensor_scalar(out=tf_[:, s, :], in0=th[:, s, :],
                                    scalar1=-MAGIC, scalar2=None,
                                    op0=mybir.AluOpType.add)
            nc.vector.scalar_tensor_tensor(out=q[:, s, :], in0=tf_[:, s, :],
                                           scalar=-2.0, in1=x[:, s, :],
                                           op0=mybir.AluOpType.mult,
                                           op1=mybir.AluOpType.add)
            nc.scalar.activation(sins[:, s, :], q[:, s, :],
                                 mybir.ActivationFunctionType.Sin,
                                 bias=0.0, scale=math.pi)
            nc.vector.tensor_tensor(out=radb[:, s, :], in0=sins[:, s, :],
                                    in1=prec[:, s, :], op=mybir.AluOpType.mult)

        # ---- persistent Z tiles (pad cols memset once) ----
        zs = [zper.tile([128, ZCOLS], bf16, name=f"z{i}") for i in range(SUPER)]
        for z in zs:
            zap = z[:]
            nc.gpsimd.memset(
                AP(zap.tensor, zap.offset + 468,
                   [list(zap.ap[0]), [FPBLK, 8], [1, FPBLK - 468]]), 0.0)
        # persistent Z^T pair tiles [128, 2, 512]
        zts = [zper.tile([128, 2, 512], bf16, name=f"zt{cp}")
               for cp in range(NCHUNK // 2)]

        def do_superblock(ebs):
            nebs = len(ebs)
            ne = nebs * 128
            yps = []
            for i, j in enumerate(ebs):
                g = gpool.tile([128, F * B32], bf16, tag=f"g{i}")
                nc.gpsimd.indirect_dma_start(
                    out=g[:], out_offset=None, in_=a2[:],
                    in_offset=bass.IndirectOffsetOnAxis(
                        ap=idx_t[:, j, 0:1], axis=0))
                nc.gpsimd.indirect_dma_start(
                    out=g[:], out_offset=None, in_=a2[:],
                    in_offset=bass.IndirectOffsetOnAxis(
                        ap=idx_t[:, j, 1:2], axis=0),
                    compute_op=mybir.AluOpType.add)
                yp = ypool.tile([128, F * B32], bf16, tag=f"yp{i}")
                # alternate Pool/DVE so radfolds don't serialize behind the
                # in-order Pool queue's gathers (and vice versa)
                eng = nc.gpsimd if i == 3 else nc.vector
                eng.tensor_tensor(
                    out=yp[:].rearrange("p (f b) -> p f b", f=F),
                    in0=g[:].rearrange("p (f b) -> p f b", f=F),
                    in1=radb[:, j, :, None].to_broadcast([128, F, B32]),
                    op=mybir.AluOpType.mult)
                yps.append(yp)

                # Z build: 9 tensor_scalar (bf16 4x) per eblock
                zap = zs[i][:]
                ypap = yp[:]
                for a in range(NSH):
                    zsl = AP(zap.tensor, zap.offset + a * BPAD,
                             [list(zap.ap[0]), [FPBLK, 8], [ABLK, 2], [1, BPAD]])
                    ysl = AP(ypap.tensor, ypap.offset,
                             [list(ypap.ap[0]), [2 * B32, 8], [B32, 2], [1, BPAD]])
                    ze = Z_ENGINE[i][a]
                    if ze == 'a':
                        nc.scalar.activation(zsl, ysl,
                                             mybir.ActivationFunctionType.Copy,
                                             bias=0.0,
                                             scale=shf[:, j, a:a + 1])
                    else:
                        eng2 = nc.gpsimd if ze == 'g' else nc.vector
                        eng2.tensor_scalar(out=zsl, in0=ysl,
                                           scalar1=shf[:, j, a:a + 1],
                                           scalar2=None,
                                           op0=mybir.AluOpType.mult)

            # transpose all chunks; copy pairs psum->sbuf
            for cp in range(NCHUNK // 2):
                pt = pst.tile([128, 2, 512], bf16, tag="pt", space="PSUM")
                for h in range(2):
                    c = 2 * cp + h
                    for i in range(nebs):
                        nc.tensor.transpose(
                            out=pt[:, h, i * 128:(i + 1) * 128],
                            in_=zs[i][:, c * 128:(c + 1) * 128],
                            identity=identb[:])
                eng = PAIR_ENGINE[cp]
                dst = zts[cp][:, :, :ne]
                src = pt[:, :, :ne]
                if eng == 'v':
                    nc.vector.tensor_copy(out=dst, in_=src)
                else:
                    nc.scalar.copy(out=dst, in_=src)

            # matmuls + po copy + out DMA
            e0 = ebs[0] * EBLK
            for fp in range(8):
                po = psm.tile([2 * NC_OUT, 512], f32, tag="po", space="PSUM")
                for pi, (r0, r1) in enumerate(
                        [(0, 128), (128, 256), (256, 384), (384, 468)]):
                    cp, h = divmod(4 * fp + pi, 2)
                    rhs = zts[cp][:, h, :ne] if r1 - r0 == 128 \
                        else zts[cp][0:84, h, :ne]
                    nc.tensor.matmul(out=po[:, :ne], lhsT=wt[(fp, pi)],
                                     rhs=rhs, start=(pi == 0), stop=(pi == 3))
                pos = opool.tile([2 * NC_OUT, 512], bf16, tag=f"pos{fp}")
                if PO_ENGINE[fp] == 'v':
                    nc.vector.tensor_copy(out=pos[:, :ne], in_=po[:, :ne])
                else:
                    nc.scalar.copy(out=pos[:, :ne], in_=po[:, :ne])
                nc.sync.dma_start(
                    out=outT[fp * 100:(fp + 1) * 100, e0:e0 + ne],
                    in_=pos[:, :ne])

        # geometry sliced: slice k covers blocks for superblocks 4k..4k+3,
        # emitted just before superblock 4(k-?) ... first slice up front,
        # later slices interleave so pipeline fill stays short
        # partial superblock (1 eblock) runs FIRST: it fills the pipeline
        # quickly and the kernel drains on a fully-pipelined superblock.
        # geometry sliced: tiny first slices, then 8-block slices emitted
        # ~2 superblocks ahead
        NSB = (NBLK - 1) // SUPER  # 12 full superblocks after the partial
        emit_geometry(NSB * SUPER, NBLK)     # block 48 only
        do_superblock([NSB * SUPER])
        emitted = 0
        for sb in range(NSB):
            if sb == 0:
                need = SUPER
            elif sb % 2 == 1:
                need = min((sb + 3) * SUPER, NSB * SUPER)
            else:
                need = emitted
            if need > emitted:
                emit_geometry(emitted, need)
                emitted = need
            do_superblock(list(range(sb * SUPER, (sb + 1) * SUPER)))

    if split_waits:
        _split_multi_waits(nc)
    return nc


def _get_nc():
    if "nc" not in _NC_CACHE:
        _NC_CACHE["nc"] = _build_bass()
    return _NC_CACHE["nc"]


# ----------------------------------------------------------------------------
# Host entry point
# ----------------------------------------------------------------------------
def kernel(atomic_descriptors, tp_weights, neighbour_displacements,
           neighbour_indices):
    atomic_descriptors = np.asarray(atomic_descriptors, dtype=np.float32)
    tp_weights = np.asarray(tp_weights, dtype=np.float32)
    neighbour_displacements = np.asarray(neighbour_displacements, dtype=np.float32)
    neighbour_indices = np.asarray(neighbour_indices, dtype=np.int32)

    # atom table: (A, 1, 25, 16) -> (A, 16, 32) f-major bf16
    A = atomic_descriptors.reshape(N_ATOMS, NB, F)
    a2 = np.zeros((N_ATOMS, F, B32), dtype=BF)
    a2[:, :, :NB] = A.transpose(0, 2, 1).astype(BF)
    a2 = a2.reshape(N_ATOMS, F * B32)

    wm = _build_weight_tensor(tp_weights).astype(BF)      # [4096, 100]
    # device layout [128, 32*100]: wmat[p, q*100+m] = wm[q*128+p, m]
    wmat = np.ascontiguousarray(
        wm.reshape(32, 128, 2 * NC_OUT).transpose(1, 0, 2)).reshape(128, -1)

    in_maps = []
    for c in range(N_CORES):
        idx_full = np.zeros((EPC, 2), dtype=np.int32)
        disp_full = np.ones((EPC, 3), dtype=np.float32)
        idx_full[:SHARD] = neighbour_indices[c * SHARD:(c + 1) * SHARD]
        disp_full[:SHARD] = neighbour_displacements[c * SHARD:(c + 1) * SHARD]
        # relayout to [128, NBLK, *]: edge j*128+p -> [p, j]
        idx2 = np.ascontiguousarray(
            idx_full.reshape(NBLK, 128, 2).transpose(1, 0, 2)).reshape(128, -1)
        disp4 = np.zeros((NBLK, 128, 4), dtype=np.float32)
        disp4[:, :, :3] = disp_full.reshape(NBLK, 128, 3)
        disp2 = np.ascontiguousarray(disp4.transpose(1, 0, 2)).reshape(128, -1)
        in_maps.append({"a2": a2, "idx": idx2, "disp": disp2, "wmat": wmat})

    nc = _get_nc()
    res = run_bass_kernel_spmd(nc, in_maps, core_ids=list(range(N_CORES)))

    out = np.empty((N_EDGES, 2, NB, F), dtype=np.float32)
    for c in range(N_CORES):
        oT = np.asarray(res.results[c]["outT"]).astype(np.float32)  # [800, EPC]
        # row fp*100 + 2*cc + df -> (f=2fp+df, par=cc//25, cm=cc%25)
        o = oT[:, :SHARD].reshape(8, 50, 2, SHARD)     # [fp, cc, df, e]
        o = o.transpose(3, 1, 0, 2).reshape(SHARD, 50, 16)  # [e, cc, f]
        o = o.reshape(SHARD, 2, 25, 16)
        out[c * SHARD:(c + 1) * SHARD] = o
    return out


if __name__ == "__main__":
    rng = np.random.default_rng(0)
    inputs = {
        "atomic_descriptors": rng.standard_normal(
            (N_ATOMS, 1, NB, F)).astype(np.float32),
        "tp_weights": (rng.standard_normal((len(PATHS), F)) * 0.1).astype(np.float32),
        "neighbour_displacements": (rng.standard_normal(
            (N_EDGES, 3)) * 1.5).astype(np.float32),
        "neighbour_indices": rng.integers(0, N_ATOMS, (N_EDGES, 2)).astype(np.int32),
    }
    out = kernel(**inputs)
    print("kernel ran, out shape", out.shape)



# revision 23
# speedup vs baseline: 1.0516x; 1.0479x over previous
"""Bond-centered tensor-moment descriptor kernel for Trainium2 (8 NeuronCores).

v2: edges sharded 8 ways; per-core pipeline per 4-eblock superblock:
  - bf16 atom table (f-major, b padded to 32); per-eblock indirect gather of
    both endpoints summed in-DMA (bf16)
  - rad fold via broadcast TT; Z[e,(fp,df,a,b26)] built with 9 tensor_scalar
    ops in full bf16 (DVE 4x mode); per-core hoisted geometry
  - PE transposes Z chunks into PSUM pairs, copies balanced over DVE/Act/Pool
  - one stationary matmul chain per f-pair; po copied to bf16 and DMAed by the
    SP queue into a transposed DRAM output [800, EPC]; host de-transposes
"""
import math
import numpy as np
import ml_dtypes

import concourse.bass as bass
from concourse import mybir
from concourse.bass import AP
from concourse.bass_utils import run_bass_kernel_spmd
from concourse.masks import make_identity
from concourse.tile import TileContext, ScopedClock

BF = ml_dtypes.bfloat16

# ----------------------------------------------------------------------------
# Problem constants
# ----------------------------------------------------------------------------
CUTOFF = 5.0
MAX_BASIS_DEG = 2
MAX_DEG = 4
N_ATOMS = 20000
N_EDGES = 50000
F = 16
N_CORES = 8

NSH = (MAX_BASIS_DEG + 1) ** 2        # 9
NB = (MAX_DEG + 1) ** 2               # 25
BPAD = 26                             # b pad in Z / W rows
B32 = 32                              # b pad in atom table (f-major rows)
NC_OUT = 2 * NB                       # 50
ABLK = NSH * BPAD                     # 234
FPBLK = 512                           # K rows per f-pair block (468 + 44 pad)
ZCOLS = 8 * FPBLK                     # 4096
NCHUNK = ZCOLS // 128                 # 32
EBLK = 128
NBLK = 49                             # ceil(6250 / 128)
EPC = NBLK * EBLK                     # 6272
SHARD = N_EDGES // N_CORES            # 6250
SUPER = 4

PATHS = [(l1, l2, l3)
         for l1 in range(MAX_BASIS_DEG + 1)
         for l2 in range(MAX_DEG + 1)
         for l3 in range(abs(l1 - l2), min(l1 + l2, MAX_DEG) + 1)]


# ----------------------------------------------------------------------------
# Clebsch-Gordan (host)
# ----------------------------------------------------------------------------
def _fac(n):
    return math.factorial(n)


def _cg(j1, m1, j2, m2, j3, m3):
    if m1 + m2 != m3:
        return 0.0
    if j3 < abs(j1 - j2) or j3 > j1 + j2:
        return 0.0
    pre = math.sqrt((2 * j3 + 1) * _fac(j3 + j1 - j2) * _fac(j3 - j1 + j2)
                    * _fac(j1 + j2 - j3) / _fac(j1 + j2 + j3 + 1))
    pre *= math.sqrt(_fac(j3 + m3) * _fac(j3 - m3) * _fac(j1 - m1) * _fac(j1 + m1)
                     * _fac(j2 - m2) * _fac(j2 + m2))
    s = 0.0
    for k in range(max(0, j2 - j3 - m1, j1 - j3 + m2),
                   min(j1 + j2 - j3, j1 - m1, j2 + m2) + 1):
        s += (-1) ** k / (_fac(k) * _fac(j1 + j2 - j3 - k) * _fac(j1 - m1 - k)
                          * _fac(j2 + m2 - k) * _fac(j3 - j2 + m1 + k)
                          * _fac(j3 - j1 - m2 + k))
    return pre * s


def _umat(l):
    U = np.zeros((2 * l + 1, 2 * l + 1), dtype=np.complex128)
    s2 = 1.0 / np.sqrt(2.0)
    for m in range(-l, l + 1):
        if m > 0:
            U[m + l, m + l] = ((-1) ** m) * s2
            U[m + l, -m + l] = s2
        elif m == 0:
            U[l, l] = 1.0
        else:
            am = -m
            U[m + l, m + l] = 1j * s2
            U[m + l, am + l] = -1j * ((-1) ** am) * s2
    return U


def _real_cg(l1, l2, l3):
    C = np.zeros((2 * l1 + 1, 2 * l2 + 1, 2 * l3 + 1), dtype=np.complex128)
    for m1 in range(-l1, l1 + 1):
        for m2 in range(-l2, l2 + 1):
            m3 = m1 + m2
            if -l3 <= m3 <= l3:
                C[m1 + l1, m2 + l2, m3 + l3] = _cg(l1, m1, l2, m2, l3, m3)
    G = np.einsum('aA,bB,cC,ABC->abc', _umat(l1), _umat(l2),
                  np.conj(_umat(l3)), C)
    G = G.real if (l1 + l2 + l3) % 2 == 0 else G.imag
    return np.ascontiguousarray(G)


def _build_weight_tensor(tp_weights):
    """[8*FPBLK, 100] stationary; row fp*512 + df*234 + a*26 + b, col 2c+df."""
    G_abc = np.zeros((NSH, NB, NC_OUT), dtype=np.float64)
    for p, (l1, l2, l3) in enumerate(PATHS):
        G = _real_cg(l1, l2, l3)
        par = (l1 + l2 + l3) % 2
        for ai in range(2 * l1 + 1):
            for bi in range(2 * l2 + 1):
                for ci in range(2 * l3 + 1):
                    v = G[ai, bi, ci]
                    if v != 0.0:
                        G_abc[l1 * l1 + ai, l2 * l2 + bi,
                              par * NB + l3 * l3 + ci] = v
    path_idx = {p: i for i, p in enumerate(PATHS)}
    l_of_a = [0, 1, 1, 1, 2, 2, 2, 2, 2]
    l_of_b = [int(np.sqrt(b)) for b in range(NB)]
    l_of_c = [int(np.sqrt(c % NB)) for c in range(NC_OUT)]

    W = np.zeros((F, NSH, BPAD, NC_OUT), dtype=np.float64)
    for ga in range(NSH):
        for gb in range(NB):
            for gc in np.nonzero(G_abc[ga, gb])[0]:
                p = path_idx[(l_of_a[ga], l_of_b[gb], l_of_c[gc])]
                for f in range(F):
                    W[f, ga, gb, gc] = G_abc[ga, gb, gc] * float(tp_weights[p, f])
    W[:, 0, :, :] *= 0.28209479177387814
    W = W.reshape(F, ABLK, NC_OUT)
    out = np.zeros((8, FPBLK, 2 * NC_OUT), dtype=np.float64)
    for fp in range(8):
        for df in range(2):
            out[fp, df * ABLK:(df + 1) * ABLK, df::2] = W[2 * fp + df]
    return out.reshape(8 * FPBLK, 2 * NC_OUT) / math.pi


# ----------------------------------------------------------------------------
# Walrus single-sync-wait patches
# ----------------------------------------------------------------------------
def _drain_and_barrier_patched(self, tick_clock, wait_clock):
    nc = self.nc
    drain_inst = nc.sync.drain()
    wait_clock.add_sem_waits(drain_inst.ins,
                             ScopedClock({None: tick_clock.global_clock}))
    si = drain_inst.ins.sync_info
    waits = list(si.on_wait) if si else []
    if len(waits) > 1:
        drain_inst.ins.sync_info = mybir.SyncInfo(on_wait=[waits[0]],
                                                  on_update=list(si.on_update))
        for w in waits[1:]:
            d2 = nc.sync.drain()
            d2.ins.sync_info = mybir.SyncInfo(on_wait=[w], on_update=[])
    nc.all_engine_barrier()
    assert self.sems is not None
    popped = nc._tile_sem_poison_stack.pop()
    assert popped is self._sem_poison
    nc.clear_and_free_semaphores(list(self.sems.allocated().values()))
    nc.all_engine_barrier()


TileContext._drain_and_barrier = _drain_and_barrier_patched


def _split_multi_waits(nc):
    for f in nc.m.functions:
        for bb in f.blocks:
            newl = []
            changed = False
            for inst in bb.instructions:
                si = inst.sync_info
                waits = list(si.on_wait) if si else []
                if len(waits) > 1:
                    changed = True
                    for k, w in enumerate(waits[:-1]):
                        nop = mybir.InstDrain(name=f"{inst.name}-sw{k}",
                                              ins=[], outs=[])
                        nop.engine = inst.engine
                        nop.sync_info = mybir.SyncInfo(on_wait=[w], on_update=[])
                        newl.append(nop)
                    inst.sync_info = mybir.SyncInfo(on_wait=[waits[-1]],
                                                    on_update=list(si.on_update))
                newl.append(inst)
            if changed:
                bb.instructions = newl


# ----------------------------------------------------------------------------
# Device kernel
# ----------------------------------------------------------------------------
_NC_CACHE = {}

# engine assignment for the 16 psum->sbuf pair copies (chunk pairs 0..15)
# and the 8 po copies, tuned for balance (gpsimd cannot touch PSUM)
PAIR_ENGINE = (['v', 'a'] * 8)
# Z-build engine per (eblock-in-superblock, a): 'v' DVE TSP, 'a' Act
# scale-AP activation, 'g' Pool TSP
Z_ENGINE = [['v'] * 9 for _ in range(4)]
PO_ENGINE = ['a'] * 8


def _build_bass(split_waits=True):
    nc = bass.Bass("TRN2", target_bir_lowering=False, debug=False)
    dt = mybir.dt
    f32 = dt.float32
    bf16 = dt.bfloat16

    a2 = nc.dram_tensor("a2", [N_ATOMS, F * B32], bf16, kind="ExternalInput").ap()
    idx = nc.dram_tensor("idx", [128, NBLK * 2], dt.int32, kind="ExternalInput").ap()
    disp = nc.dram_tensor("disp", [128, NBLK * 4], f32, kind="ExternalInput").ap()
    wmat = nc.dram_tensor("wmat", [128, 32 * 2 * NC_OUT], bf16,
                          kind="ExternalInput").ap()
    outT = nc.dram_tensor("outT", [8 * 2 * NC_OUT, EPC], bf16,
                          kind="ExternalOutput").ap()

    from contextlib import ExitStack
    with TileContext(nc) as tc, ExitStack() as ctx:
        consts = ctx.enter_context(tc.tile_pool(name="consts", bufs=1))
        wpool = ctx.enter_context(tc.tile_pool(name="wpool", bufs=1))
        geom = ctx.enter_context(tc.tile_pool(name="geom", bufs=1))
        zper = ctx.enter_context(tc.tile_pool(name="zper", bufs=1))   # Z + zts persistent
        gpool = ctx.enter_context(tc.tile_pool(name="gpool", bufs=3))  # gathered y
        opool = ctx.enter_context(tc.tile_pool(name="opool", bufs=2))  # po sbuf bf16
        pst = ctx.enter_context(tc.tile_pool(name="pst", bufs=5, space="PSUM"))
        psm = ctx.enter_context(tc.tile_pool(name="psm", bufs=3, space="PSUM"))

        # ---- constants ----
        identb = consts.tile([128, 128], bf16)
        make_identity(nc, identb[:])
        krow = consts.tile([128, F], f32)
        kint = consts.tile([128, F], dt.int32)
        nc.gpsimd.iota(kint[:], pattern=[[1, F]], base=1, channel_multiplier=0)
        nc.vector.tensor_copy(out=krow[:], in_=kint[:])

        # ---- hoisted inputs: idx first (gathers need it), then disp
        # (geometry), then the big weight DMA (needed latest, ~8us in) ----
        idx_t = geom.tile([128, NBLK, 2], dt.int32)
        nc.sync.dma_start(out=idx_t[:], in_=idx[:, :])
        disp_t = geom.tile([128, NBLK, 4], f32)
        nc.sync.dma_start(out=disp_t[:], in_=disp[:, :])

        PIECES = [(0, 128), (128, 256), (256, 384), (384, 468)]
        wbig = wpool.tile([128, 32, 2 * NC_OUT], bf16, name="wbig")
        nc.sync.dma_start(
            out=wbig[:].rearrange("p q m -> p (q m)"), in_=wmat[:, :])
        wt = {(fp, pi): (wbig[:, 4 * fp + pi, :] if pi < 3
                         else wbig[0:84, 4 * fp + pi, :])
              for fp in range(8) for pi in range(4)}

        NJ = NBLK  # 49
        sq = geom.tile([128, NJ, 3], f32)
        r2 = geom.tile([128, NJ], f32)
        r = geom.tile([128, NJ], f32)
        rm = geom.tile([128, NJ], f32)
        rinv = geom.tile([128, NJ], f32)
        u = geom.tile([128, NJ, 3], f32)
        shf = geom.tile([128, NJ, NSH], f32)
        t6 = geom.tile([128, NJ], f32)
        t8 = geom.tile([128, NJ], f32)
        t8b = geom.tile([128, NJ], f32)
        rc = geom.tile([128, NJ], f32)
        x = geom.tile([128, NJ, F], f32)
        prec = geom.tile([128, NJ, F], f32)
        th = geom.tile([128, NJ, F], f32)
        tf_ = geom.tile([128, NJ, F], f32)
        q = geom.tile([128, NJ, F], f32)
        sins = geom.tile([128, NJ, F], f32)
        radb = geom.tile([128, NJ, F], bf16)

        def emit_geometry(j0, j1, sh_eng=None):
            """Geometry chain for blocks [j0, j1) — sliced to overlap pipeline."""
            s = slice(j0, j1)
            nj = j1 - j0
            sh_eng_ = sh_eng or nc.vector
            nc.scalar.square(sq[:, s, :], disp_t[:, s, 0:3])
            nc.vector.tensor_reduce(out=r2[:, s], in_=sq[:, s, :],
                                    op=mybir.AluOpType.add,
                                    axis=mybir.AxisListType.X)
            nc.scalar.sqrt(r[:, s], r2[:, s])
            nc.vector.tensor_scalar(out=rm[:, s], in0=r[:, s], scalar1=1e-9,
                                    scalar2=None, op0=mybir.AluOpType.max)
            nc.vector.reciprocal(rinv[:, s], rm[:, s])
            nc.vector.tensor_tensor(
                out=u[:, s, :], in0=disp_t[:, s, 0:3],
                in1=rinv[:, s, None].to_broadcast([128, nj, 3]),
                op=mybir.AluOpType.mult)
            # rc = min(r/C, 1): sinc(k*1) == 0 for integer k, so the
            # cutoff mask is implicit in the clamp
            nc.vector.tensor_scalar(out=rc[:, s], in0=rm[:, s],
                                    scalar1=1.0 / CUTOFF, scalar2=1.0,
                                    op0=mybir.AluOpType.mult,
                                    op1=mybir.AluOpType.min)
            nc.vector.tensor_tensor(
                out=x[:, s, :],
                in0=rc[:, s, None].to_broadcast([128, nj, F]),
                in1=krow[:, None, :].to_broadcast([128, nj, F]),
                op=mybir.AluOpType.mult)
            nc.vector.reciprocal(prec[:, s, :], x[:, s, :])
            MAGIC = 8388608.0
            nc.vector.tensor_scalar(out=th[:, s, :], in0=x[:, s, :], scalar1=0.5,
                                    scalar2=MAGIC, op0=mybir.AluOpType.mult,
                                    op1=mybir.AluOpType.add)
            nc.vector.tensor_scalar(out=tf_[:, s, :], in0=th[:, s, :],
                                    scalar1=-MAGIC, scalar2=None,
                                    op0=mybir.AluOpType.add)
            nc.vector.scalar_tensor_tensor(out=q[:, s, :], in0=tf_[:, s, :],
                                           scalar=-2.0, in1=x[:, s, :],
                                           op0=mybir.AluOpType.mult,
                                           op1=mybir.AluOpType.add)
            nc.scalar.activation(sins[:, s, :], q[:, s, :],
                                 mybir.ActivationFunctionType.Sin,
                                 bias=0.0, scale=math.pi)
            nc.vector.tensor_tensor(out=radb[:, s, :], in0=sins[:, s, :],
                                    in1=prec[:, s, :], op=mybir.AluOpType.mult)
            c1 = 0.4886025119029199
            c2 = 1.0925484305920792
            ux, uy, uz = u[:, s, 0:1], u[:, s, 1:2], u[:, s, 2:3]
            sh_eng_.memset(shf[:, s, 0:1], 0.28209479177387814)
            sh_eng_.tensor_scalar(out=shf[:, s, 1:2], in0=uy, scalar1=c1,
                                    scalar2=None, op0=mybir.AluOpType.mult)
            sh_eng_.tensor_scalar(out=shf[:, s, 2:3], in0=uz, scalar1=c1,
                                    scalar2=None, op0=mybir.AluOpType.mult)
            sh_eng_.tensor_scalar(out=shf[:, s, 3:4], in0=ux, scalar1=c1,
                                    scalar2=None, op0=mybir.AluOpType.mult)
            sh_eng_.scalar_tensor_tensor(out=shf[:, s, 4:5], in0=ux, scalar=c2,
                                           in1=uy, op0=mybir.AluOpType.mult,
                                           op1=mybir.AluOpType.mult)
            sh_eng_.scalar_tensor_tensor(out=shf[:, s, 5:6], in0=uy, scalar=c2,
                                           in1=uz, op0=mybir.AluOpType.mult,
                                           op1=mybir.AluOpType.mult)
            sh_eng_.scalar_tensor_tensor(out=t6[:, s, None], in0=uz, scalar=3.0,
                                           in1=uz, op0=mybir.AluOpType.mult,
                                           op1=mybir.AluOpType.mult)
            nc.scalar.activation(shf[:, s, 6:7], t6[:, s, None],
                                 mybir.ActivationFunctionType.Copy,
                                 bias=-0.31539156525252005,
                                 scale=0.31539156525252005)
            sh_eng_.scalar_tensor_tensor(out=shf[:, s, 7:8], in0=ux, scalar=c2,
                                           in1=uz, op0=mybir.AluOpType.mult,
                                           op1=mybir.AluOpType.mult)
            sh_eng_.scalar_tensor_tensor(out=t8[:, s, None], in0=ux,
                                           scalar=0.5 * c2, in1=ux,
                                           op0=mybir.AluOpType.mult,
                                           op1=mybir.AluOpType.mult)
            sh_eng_.scalar_tensor_tensor(out=t8b[:, s, None], in0=uy,
                                           scalar=-0.5 * c2, in1=uy,
                                           op0=mybir.AluOpType.mult,
                                           op1=mybir.AluOpType.mult)
            sh_eng_.tensor_add(out=shf[:, s, 8:9], in0=t8[:, s, None],
                                 in1=t8b[:, s, None])
            # rad = sinc(k r / C) * mask  (bf16)

        # ---- persistent Z tiles, double-buffered across superblocks ----
        zs2 = [[zper.tile([128, ZCOLS], bf16, name=f"z{d}_{i}")
                for i in range(SUPER)] for d in range(2)]
        for zset in zs2:
            for z in zset:
                zap = z[:]
                nc.vector.memset(
                    AP(zap.tensor, zap.offset + 468,
                       [list(zap.ap[0]), [FPBLK, 8], [1, FPBLK - 468]]), 0.0)
        # persistent Z^T pair tiles [128, 2, 512], double-buffered
        zts2 = [[zper.tile([128, 2, 512], bf16, name=f"zt{d}_{cp}")
                 for cp in range(NCHUNK // 2)] for d in range(2)]

        sb_count = [0]

        def do_superblock(ebs, pair_map=None, po_map=None):
            zs = zs2[sb_count[0] % 2]
            zts = zts2[sb_count[0] % 2]
            sb_count[0] += 1
            nebs = len(ebs)
            ne = nebs * 128
            for i, j in enumerate(ebs):
                g = gpool.tile([128, F * B32], bf16, tag=f"g{i}")
                nc.gpsimd.indirect_dma_start(
                    out=g[:], out_offset=None, in_=a2[:],
                    in_offset=bass.IndirectOffsetOnAxis(
                        ap=idx_t[:, j, 0:1], axis=0))
                nc.gpsimd.indirect_dma_start(
                    out=g[:], out_offset=None, in_=a2[:],
                    in_offset=bass.IndirectOffsetOnAxis(
                        ap=idx_t[:, j, 1:2], axis=0),
                    compute_op=mybir.AluOpType.add)
                # radfold writes the rad-scaled features straight into the
                # Z a=0 slice (sh_0 const is folded into W host-side);
                # alternate Pool/DVE so radfolds don't serialize behind the
                # in-order Pool queue's gathers (and vice versa)
                zap = zs[i][:]
                gap_ = g[:]
                a0sl = AP(zap.tensor, zap.offset,
                          [list(zap.ap[0]), [FPBLK, 8], [ABLK, 2], [1, BPAD]])
                gsl = AP(gap_.tensor, gap_.offset,
                         [list(gap_.ap[0]), [2 * B32, 8], [B32, 2], [1, BPAD]])
                radsl = AP(radb[:].tensor,
                           radb[:].offset + j * F,
                           [list(radb[:].ap[0]), [2, 8], [1, 2], [0, BPAD]])
                eng = nc.gpsimd if i == 3 else nc.vector
                eng.tensor_tensor(out=a0sl, in0=gsl, in1=radsl,
                                  op=mybir.AluOpType.mult)

                # Z build: 8 tensor_scalar (bf16 4x) per eblock off the
                # a0 slice
                for a in range(1, NSH):
                    zsl = AP(zap.tensor, zap.offset + a * BPAD,
                             [list(zap.ap[0]), [FPBLK, 8], [ABLK, 2], [1, BPAD]])
                    ze = Z_ENGINE[i][a]
                    if ze == 'a':
                        nc.scalar.activation(zsl, a0sl,
                                             mybir.ActivationFunctionType.Copy,
                                             bias=0.0,
                                             scale=shf[:, j, a:a + 1])
                    else:
                        eng2 = nc.gpsimd if ze == 'g' else nc.vector
                        eng2.tensor_scalar(out=zsl, in0=a0sl,
                                           scalar1=shf[:, j, a:a + 1],
                                           scalar2=None,
                                           op0=mybir.AluOpType.mult)

            # transpose all chunks; copy pairs psum->sbuf
            for cp in range(NCHUNK // 2):
                pt = pst.tile([128, 2, 512], bf16, tag="pt", space="PSUM")
                for h in range(2):
                    c = 2 * cp + h
                    for i in range(nebs):
                        nc.tensor.transpose(
                            out=pt[:, h, i * 128:(i + 1) * 128],
                            in_=zs[i][:, c * 128:(c + 1) * 128],
                            identity=identb[:])
                eng = (pair_map or PAIR_ENGINE)[cp]
                dst = zts[cp][:, :, :ne]
                src = pt[:, :, :ne]
                if eng == 'v':
                    nc.vector.tensor_copy(out=dst, in_=src)
                else:
                    nc.scalar.copy(out=dst, in_=src)

            # matmuls + po copy + out DMA
            e0 = ebs[0] * EBLK
            for fp in range(8):
                po = psm.tile([2 * NC_OUT, 512], f32, tag="po", space="PSUM")
                for pi, (r0, r1) in enumerate(
                        [(0, 128), (128, 256), (256, 384), (384, 468)]):
                    cp, h = divmod(4 * fp + pi, 2)
                    rhs = zts[cp][:, h, :ne] if r1 - r0 == 128 \
                        else zts[cp][0:84, h, :ne]
                    nc.tensor.matmul(out=po[:, :ne], lhsT=wt[(fp, pi)],
                                     rhs=rhs, start=(pi == 0), stop=(pi == 3))
                pos = opool.tile([2 * NC_OUT, 512], bf16, tag=f"pos{fp}")
                if (po_map or PO_ENGINE)[fp] == 'v':
                    nc.vector.tensor_copy(out=pos[:, :ne], in_=po[:, :ne])
                else:
                    nc.scalar.copy(out=pos[:, :ne], in_=po[:, :ne])
                nc.sync.dma_start(
                    out=outT[fp * 100:(fp + 1) * 100, e0:e0 + ne],
                    in_=pos[:, :ne])

        # geometry sliced: slice k covers blocks for superblocks 4k..4k+3,
        # emitted just before superblock 4(k-?) ... first slice up front,
        # later slices interleave so pipeline fill stays short
        # partial superblock (1 eblock) runs FIRST: it fills the pipeline
        # quickly and the kernel drains on a fully-pipelined superblock.
        # geometry sliced: tiny first slices, then 8-block slices emitted
        # ~2 superblocks ahead
        NSB = (NBLK - 1) // SUPER  # 12 full superblocks after the partial
        emit_geometry(NSB * SUPER, NBLK)     # block 48 only
        do_superblock([NSB * SUPER])
        emitted = 0
        for sb in range(NSB):
            if sb == 0:
                need = SUPER
            elif sb % 2 == 1:
                need = min((sb + 3) * SUPER, NSB * SUPER)
            else:
                need = emitted
            if need > emitted:
                emit_geometry(emitted, need)
                emitted = need
            last = sb == NSB - 1
            do_superblock(list(range(sb * SUPER, (sb + 1) * SUPER)),
                          po_map=['v'] * 8 if last else None)

    if split_waits:
        _split_multi_waits(nc)
    return nc


def _get_nc():
    if "nc" not in _NC_CACHE:
        _NC_CACHE["nc"] = _build_bass()
    return _NC_CACHE["nc"]


# ----------------------------------------------------------------------------
# Host entry point
# ----------------------------------------------------------------------------
def kernel(atomic_descriptors, tp_weights, neighbour_displacements,
           neighbour_indices):
    atomic_descriptors = np.asarray(atomic_descriptors, dtype=np.float32)
    tp_weights = np.asarray(tp_weights, dtype=np.float32)
    neighbour_displacements = np.asarray(neighbour_displacements, dtype=np.float32)
    neighbour_indices = np.asarray(neighbour_indices, dtype=np.int32)

    # atom table: (A, 1, 25, 16) -> (A, 16, 32) f-major bf16
    A = atomic_descriptors.reshape(N_ATOMS, NB, F)
    a2 = np.zeros((N_ATOMS, F, B32), dtype=BF)
    a2[:, :, :NB] = A.transpose(0, 2, 1).astype(BF)
    a2 = a2.reshape(N_ATOMS, F * B32)

    wm = _build_weight_tensor(tp_weights).astype(BF)      # [4096, 100]
    # device layout [128, 32*100]: wmat[p, q*100+m] = wm[q*128+p, m]
    wmat = np.ascontiguousarray(
        wm.reshape(32, 128, 2 * NC_OUT).transpose(1, 0, 2)).reshape(128, -1)

    in_maps = []
    for c in range(N_CORES):
        idx_full = np.zeros((EPC, 2), dtype=np.int32)
        disp_full = np.ones((EPC, 3), dtype=np.float32)
        idx_full[:SHARD] = neighbour_indices[c * SHARD:(c + 1) * SHARD]
        disp_full[:SHARD] = neighbour_displacements[c * SHARD:(c + 1) * SHARD]
        # relayout to [128, NBLK, *]: edge j*128+p -> [p, j]
        idx2 = np.ascontiguousarray(
            idx_full.reshape(NBLK, 128, 2).transpose(1, 0, 2)).reshape(128, -1)
        disp4 = np.zeros((NBLK, 128, 4), dtype=np.float32)
        disp4[:, :, :3] = disp_full.reshape(NBLK, 128, 3)
        disp2 = np.ascontiguousarray(disp4.transpose(1, 0, 2)).reshape(128, -1)
        in_maps.append({"a2": a2, "idx": idx2, "disp": disp2, "wmat": wmat})

    nc = _get_nc()
    res = run_bass_kernel_spmd(nc, in_maps, core_ids=list(range(N_CORES)))

    out = np.empty((N_EDGES, 2, NB, F), dtype=np.float32)
    for c in range(N_CORES):
        oT = np.asarray(res.results[c]["outT"]).astype(np.float32)  # [800, EPC]
        # row fp*100 + 2*cc + df -> (f=2fp+df, par=cc//25, cm=cc%25)
        o = oT[:, :SHARD].reshape(8, 50, 2, SHARD)     # [fp, cc, df, e]
        o = o.transpose(3, 1, 0, 2).reshape(SHARD, 50, 16)  # [e, cc, f]
        o = o.reshape(SHARD, 2, 25, 16)
        out[c * SHARD:(c + 1) * SHARD] = o
    return out


if __name__ == "__main__":
    rng = np.random.default_rng(0)
    inputs = {
        "atomic_descriptors": rng.standard_normal(
            (N_ATOMS, 1, NB, F)).astype(np.float32),
        "tp_weights": (rng.standard_normal((len(PATHS), F)) * 0.1).astype(np.float32),
        "neighbour_displacements": (rng.standard_normal(
            (N_EDGES, 3)) * 1.5).astype(np.float32),
        "neighbour_indices": rng.integers(0, N_ATOMS, (N_EDGES, 2)).astype(np.int32),
    }
    out = kernel(**inputs)
    print("kernel ran, out shape", out.shape)



# revision 47
# speedup vs baseline: 1.0907x; 1.0372x over previous
"""Bond-centered tensor-moment descriptor kernel for Trainium2 (8 NeuronCores).

v3: edges sharded 8 ways; per-core pipeline per 4-eblock superblock:
  - bf16 atom table (f-major, b padded to 32); per-eblock indirect gather of
    both endpoints summed in-DMA (bf16); input DMAs ordered idx/disp/wmat so
    the gathers and geometry start earliest
  - rad fold written directly into the Z a=0 slice (sh_0 and 1/pi folded into
    W host-side); cutoff mask folded into rc=min(r/C,1) since sinc(k)=0;
    remaining 8 a-slices built with tensor_scalar off the a0 slice (DVE 4x)
  - PE transposes Z chunks into PSUM pairs; pair/po copies balanced over
    DVE/Act with per-superblock phase rotation; zs/zts double-buffered
    across superblocks to break WAR coupling
  - one stationary matmul chain per f-pair; po copied to bf16 and DMAed by the
    SP queue into a transposed DRAM output [800, EPC]; host de-transposes
"""
import math
import numpy as np
import ml_dtypes

import concourse.bass as bass
from concourse import mybir
from concourse.bass import AP
from concourse.bass_utils import run_bass_kernel_spmd
from concourse.masks import make_identity
from concourse.tile import TileContext, ScopedClock

BF = ml_dtypes.bfloat16

# ----------------------------------------------------------------------------
# Problem constants
# ----------------------------------------------------------------------------
CUTOFF = 5.0
MAX_BASIS_DEG = 2
MAX_DEG = 4
N_ATOMS = 20000
N_EDGES = 50000
F = 16
N_CORES = 8

NSH = (MAX_BASIS_DEG + 1) ** 2        # 9
NB = (MAX_DEG + 1) ** 2               # 25
BPAD = 26                             # b pad in Z / W rows
B32 = 32                              # b pad in atom table (f-major rows)
NC_OUT = 2 * NB                       # 50
ABLK = NSH * BPAD                     # 234
FPBLK = 512                           # K rows per f-pair block (468 + 44 pad)
ZCOLS = 8 * FPBLK                     # 4096
NCHUNK = ZCOLS // 128                 # 32
EBLK = 128
NBLK = 49                             # ceil(6250 / 128)
EPC = NBLK * EBLK                     # 6272
SHARD = N_EDGES // N_CORES            # 6250
SUPER = 4

PATHS = [(l1, l2, l3)
         for l1 in range(MAX_BASIS_DEG + 1)
         for l2 in range(MAX_DEG + 1)
         for l3 in range(abs(l1 - l2), min(l1 + l2, MAX_DEG) + 1)]


# ----------------------------------------------------------------------------
# Clebsch-Gordan (host)
# ----------------------------------------------------------------------------
def _fac(n):
    return math.factorial(n)


def _cg(j1, m1, j2, m2, j3, m3):
    if m1 + m2 != m3:
        return 0.0
    if j3 < abs(j1 - j2) or j3 > j1 + j2:
        return 0.0
    pre = math.sqrt((2 * j3 + 1) * _fac(j3 + j1 - j2) * _fac(j3 - j1 + j2)
                    * _fac(j1 + j2 - j3) / _fac(j1 + j2 + j3 + 1))
    pre *= math.sqrt(_fac(j3 + m3) * _fac(j3 - m3) * _fac(j1 - m1) * _fac(j1 + m1)
                     * _fac(j2 - m2) * _fac(j2 + m2))
    s = 0.0
    for k in range(max(0, j2 - j3 - m1, j1 - j3 + m2),
                   min(j1 + j2 - j3, j1 - m1, j2 + m2) + 1):
        s += (-1) ** k / (_fac(k) * _fac(j1 + j2 - j3 - k) * _fac(j1 - m1 - k)
                          * _fac(j2 + m2 - k) * _fac(j3 - j2 + m1 + k)
                          * _fac(j3 - j1 - m2 + k))
    return pre * s


def _umat(l):
    U = np.zeros((2 * l + 1, 2 * l + 1), dtype=np.complex128)
    s2 = 1.0 / np.sqrt(2.0)
    for m in range(-l, l + 1):
        if m > 0:
            U[m + l, m + l] = ((-1) ** m) * s2
            U[m + l, -m + l] = s2
        elif m == 0:
            U[l, l] = 1.0
        else:
            am = -m
            U[m + l, m + l] = 1j * s2
            U[m + l, am + l] = -1j * ((-1) ** am) * s2
    return U


def _real_cg(l1, l2, l3):
    C = np.zeros((2 * l1 + 1, 2 * l2 + 1, 2 * l3 + 1), dtype=np.complex128)
    for m1 in range(-l1, l1 + 1):
        for m2 in range(-l2, l2 + 1):
            m3 = m1 + m2
            if -l3 <= m3 <= l3:
                C[m1 + l1, m2 + l2, m3 + l3] = _cg(l1, m1, l2, m2, l3, m3)
    G = np.einsum('aA,bB,cC,ABC->abc', _umat(l1), _umat(l2),
                  np.conj(_umat(l3)), C)
    G = G.real if (l1 + l2 + l3) % 2 == 0 else G.imag
    return np.ascontiguousarray(G)


def _build_weight_tensor(tp_weights):
    """[8*FPBLK, 100] stationary; row fp*512 + df*234 + a*26 + b, col 2c+df."""
    G_abc = np.zeros((NSH, NB, NC_OUT), dtype=np.float64)
    for p, (l1, l2, l3) in enumerate(PATHS):
        G = _real_cg(l1, l2, l3)
        par = (l1 + l2 + l3) % 2
        for ai in range(2 * l1 + 1):
            for bi in range(2 * l2 + 1):
                for ci in range(2 * l3 + 1):
                    v = G[ai, bi, ci]
                    if v != 0.0:
                        G_abc[l1 * l1 + ai, l2 * l2 + bi,
                              par * NB + l3 * l3 + ci] = v
    path_idx = {p: i for i, p in enumerate(PATHS)}
    l_of_a = [0, 1, 1, 1, 2, 2, 2, 2, 2]
    l_of_b = [int(np.sqrt(b)) for b in range(NB)]
    l_of_c = [int(np.sqrt(c % NB)) for c in range(NC_OUT)]

    W = np.zeros((F, NSH, BPAD, NC_OUT), dtype=np.float64)
    for ga in range(NSH):
        for gb in range(NB):
            for gc in np.nonzero(G_abc[ga, gb])[0]:
                p = path_idx[(l_of_a[ga], l_of_b[gb], l_of_c[gc])]
                for f in range(F):
                    W[f, ga, gb, gc] = G_abc[ga, gb, gc] * float(tp_weights[p, f])
    W[:, 0, :, :] *= 0.28209479177387814
    W /= np.arange(1, F + 1, dtype=np.float64)[:, None, None, None]
    W = W.reshape(F, ABLK, NC_OUT)
    out = np.zeros((8, FPBLK, 2 * NC_OUT), dtype=np.float64)
    for fp in range(8):
        for df in range(2):
            out[fp, df * ABLK:(df + 1) * ABLK, df::2] = W[2 * fp + df]
    return out.reshape(8 * FPBLK, 2 * NC_OUT) / math.pi


# ----------------------------------------------------------------------------
# Walrus single-sync-wait patches
# ----------------------------------------------------------------------------
def _drain_and_barrier_patched(self, tick_clock, wait_clock):
    nc = self.nc
    drain_inst = nc.sync.drain()
    wait_clock.add_sem_waits(drain_inst.ins,
                             ScopedClock({None: tick_clock.global_clock}))
    si = drain_inst.ins.sync_info
    waits = list(si.on_wait) if si else []
    if len(waits) > 1:
        drain_inst.ins.sync_info = mybir.SyncInfo(on_wait=[waits[0]],
                                                  on_update=list(si.on_update))
        for w in waits[1:]:
            d2 = nc.sync.drain()
            d2.ins.sync_info = mybir.SyncInfo(on_wait=[w], on_update=[])
    nc.all_engine_barrier()
    assert self.sems is not None
    popped = nc._tile_sem_poison_stack.pop()
    assert popped is self._sem_poison
    nc.clear_and_free_semaphores(list(self.sems.allocated().values()))
    nc.all_engine_barrier()


TileContext._drain_and_barrier = _drain_and_barrier_patched


def _split_multi_waits(nc):
    for f in nc.m.functions:
        for bb in f.blocks:
            newl = []
            changed = False
            for inst in bb.instructions:
                si = inst.sync_info
                waits = list(si.on_wait) if si else []
                if len(waits) > 1:
                    changed = True
                    for k, w in enumerate(waits[:-1]):
                        nop = mybir.InstDrain(name=f"{inst.name}-sw{k}",
                                              ins=[], outs=[])
                        nop.engine = inst.engine
                        nop.sync_info = mybir.SyncInfo(on_wait=[w], on_update=[])
                        newl.append(nop)
                    inst.sync_info = mybir.SyncInfo(on_wait=[waits[-1]],
                                                    on_update=list(si.on_update))
                newl.append(inst)
            if changed:
                bb.instructions = newl


# ----------------------------------------------------------------------------
# Device kernel
# ----------------------------------------------------------------------------
_NC_CACHE = {}

# engine assignment for the 16 psum->sbuf pair copies (chunk pairs 0..15)
# and the 8 po copies, tuned for balance (gpsimd cannot touch PSUM)
PAIR_ENGINE = (['v', 'a'] * 8)
# Z-build engine per (eblock-in-superblock, a): 'v' DVE TSP, 'a' Act
# scale-AP activation, 'g' Pool TSP
Z_ENGINE = [['v'] * 9 for _ in range(4)]
PO_ENGINE = ['v'] + ['a'] * 7


def _build_bass(split_waits=True):
    nc = bass.Bass("TRN2", target_bir_lowering=False, debug=False)
    dt = mybir.dt
    f32 = dt.float32
    bf16 = dt.bfloat16

    a2 = nc.dram_tensor("a2", [N_ATOMS, F * B32], bf16, kind="ExternalInput").ap()
    idx = nc.dram_tensor("idx", [128, NBLK * 2], dt.int32, kind="ExternalInput").ap()
    disp = nc.dram_tensor("disp", [128, NBLK * 4], f32, kind="ExternalInput").ap()
    wmat = nc.dram_tensor("wmat", [128, 32 * 2 * NC_OUT], bf16,
                          kind="ExternalInput").ap()
    outT = nc.dram_tensor("outT", [8 * 2 * NC_OUT, EPC], bf16,
                          kind="ExternalOutput").ap()

    from contextlib import ExitStack
    with TileContext(nc) as tc, ExitStack() as ctx:
        consts = ctx.enter_context(tc.tile_pool(name="consts", bufs=1))
        wpool = ctx.enter_context(tc.tile_pool(name="wpool", bufs=1))
        geom = ctx.enter_context(tc.tile_pool(name="geom", bufs=1))
        zper = ctx.enter_context(tc.tile_pool(name="zper", bufs=1))   # Z + zts persistent
        gpool = ctx.enter_context(tc.tile_pool(name="gpool", bufs=2))  # gathered y
        opool = ctx.enter_context(tc.tile_pool(name="opool", bufs=2))  # po sbuf bf16
        pst = ctx.enter_context(tc.tile_pool(name="pst", bufs=5, space="PSUM"))
        psm = ctx.enter_context(tc.tile_pool(name="psm", bufs=3, space="PSUM"))

        # ---- constants ----
        identb = consts.tile([128, 128], bf16)
        make_identity(nc, identb[:])

        krow = consts.tile([128, F], f32)
        kint = consts.tile([128, F], dt.int32)
        nc.gpsimd.iota(kint[:], pattern=[[1, F]], base=1, channel_multiplier=0)
        nc.vector.tensor_copy(out=krow[:], in_=kint[:])

        # ---- hoisted inputs: block-48 disp slice first (shortest pole for
        # the first geometry chain), then idx (gathers), then full disp,
        # then the big weight DMA (needed latest, ~8us in) ----
        idx_t = geom.tile([128, NBLK, 2], dt.int32)
        nc.sync.dma_start(out=idx_t[:], in_=idx[:, :])
        disp_t = geom.tile([128, NBLK, 4], f32)
        nc.sync.dma_start(out=disp_t[:, 48, :], in_=disp[:, 4 * 48:4 * 49])
        nc.sync.dma_start(out=disp_t[:, 0:48, :], in_=disp[:, 0:4 * 48])

        PIECES = [(0, 128), (128, 256), (256, 384), (384, 468)]
        wbig = wpool.tile([128, 32, 2 * NC_OUT], bf16, name="wbig")
        nc.sync.dma_start(
            out=wbig[:].rearrange("p q m -> p (q m)"), in_=wmat[:, :])
        wt = {(fp, pi): (wbig[:, 4 * fp + pi, :] if pi < 3
                         else wbig[0:84, 4 * fp + pi, :])
              for fp in range(8) for pi in range(4)}

        NJ = NBLK  # 49
        sq = geom.tile([128, NJ, 3], f32)
        r2 = geom.tile([128, NJ], f32)
        r = geom.tile([128, NJ], f32)
        rm = geom.tile([128, NJ], f32)
        rinv = geom.tile([128, NJ], f32)
        u = geom.tile([128, NJ, 3], f32)
        shf = geom.tile([128, NJ, NSH], f32)
        t6 = geom.tile([128, NJ], f32)
        t8 = geom.tile([128, NJ], f32)
        t8b = geom.tile([128, NJ], f32)
        rc = geom.tile([128, NJ], f32)
        x = geom.tile([128, NJ, F], f32)
        rcinv = geom.tile([128, NJ], f32)
        th = geom.tile([128, NJ, F], f32)
        tf_ = geom.tile([128, NJ, F], f32)
        q = geom.tile([128, NJ, F], f32)
        sins = geom.tile([128, NJ, F], f32)
        radb = geom.tile([128, NJ, F], bf16)

        def emit_geometry(j0, j1, sh_eng=None):
            """Geometry chain for blocks [j0, j1) — sliced to overlap pipeline."""
            s = slice(j0, j1)
            nj = j1 - j0
            sh_eng_ = sh_eng or nc.vector
            nc.scalar.square(sq[:, s, :], disp_t[:, s, 0:3])
            nc.vector.tensor_reduce(out=r2[:, s], in_=sq[:, s, :],
                                    op=mybir.AluOpType.add,
                                    axis=mybir.AxisListType.X)
            nc.scalar.sqrt(r[:, s], r2[:, s])
            nc.vector.tensor_scalar(out=rm[:, s], in0=r[:, s], scalar1=1e-9,
                                    scalar2=None, op0=mybir.AluOpType.max)
            nc.vector.reciprocal(rinv[:, s], rm[:, s])
            nc.vector.tensor_tensor(
                out=u[:, s, :], in0=disp_t[:, s, 0:3],
                in1=rinv[:, s, None].to_broadcast([128, nj, 3]),
                op=mybir.AluOpType.mult)
            # rc = min(r/C, 1): sinc(k*1) == 0 for integer k, so the
            # cutoff mask is implicit in the clamp
            nc.vector.tensor_scalar(out=rc[:, s], in0=rm[:, s],
                                    scalar1=1.0 / CUTOFF, scalar2=1.0,
                                    op0=mybir.AluOpType.mult,
                                    op1=mybir.AluOpType.min)
            nc.vector.reciprocal(rcinv[:, s], rc[:, s])
            nc.vector.tensor_tensor(
                out=x[:, s, :],
                in0=rc[:, s, None].to_broadcast([128, nj, F]),
                in1=krow[:, None, :].to_broadcast([128, nj, F]),
                op=mybir.AluOpType.mult)

            MAGIC = 8388608.0
            nc.vector.tensor_scalar(out=th[:, s, :], in0=x[:, s, :], scalar1=0.5,
                                    scalar2=MAGIC, op0=mybir.AluOpType.mult,
                                    op1=mybir.AluOpType.add)
            nc.vector.tensor_scalar(out=tf_[:, s, :], in0=th[:, s, :],
                                    scalar1=-MAGIC, scalar2=None,
                                    op0=mybir.AluOpType.add)
            nc.vector.scalar_tensor_tensor(out=q[:, s, :], in0=tf_[:, s, :],
                                           scalar=-2.0, in1=x[:, s, :],
                                           op0=mybir.AluOpType.mult,
                                           op1=mybir.AluOpType.add)
            nc.scalar.activation(sins[:, s, :], q[:, s, :],
                                 mybir.ActivationFunctionType.Sin,
                                 bias=0.0, scale=math.pi)
            nc.vector.tensor_tensor(
                out=radb[:, s, :], in0=sins[:, s, :],
                in1=rcinv[:, s, None].to_broadcast([128, nj, F]),
                op=mybir.AluOpType.mult)
            c1 = 0.4886025119029199
            c2 = 1.0925484305920792
            ux, uy, uz = u[:, s, 0:1], u[:, s, 1:2], u[:, s, 2:3]
            sh_eng_.memset(shf[:, s, 0:1], 0.28209479177387814)
            sh_eng_.tensor_scalar(out=shf[:, s, 1:2], in0=uy, scalar1=c1,
                                    scalar2=None, op0=mybir.AluOpType.mult)
            sh_eng_.tensor_scalar(out=shf[:, s, 2:3], in0=uz, scalar1=c1,
                                    scalar2=None, op0=mybir.AluOpType.mult)
            sh_eng_.tensor_scalar(out=shf[:, s, 3:4], in0=ux, scalar1=c1,
                                    scalar2=None, op0=mybir.AluOpType.mult)
            sh_eng_.scalar_tensor_tensor(out=shf[:, s, 4:5], in0=ux, scalar=c2,
                                           in1=uy, op0=mybir.AluOpType.mult,
                                           op1=mybir.AluOpType.mult)
            sh_eng_.scalar_tensor_tensor(out=shf[:, s, 5:6], in0=uy, scalar=c2,
                                           in1=uz, op0=mybir.AluOpType.mult,
                                           op1=mybir.AluOpType.mult)
            sh_eng_.scalar_tensor_tensor(out=t6[:, s, None], in0=uz, scalar=3.0,
                                           in1=uz, op0=mybir.AluOpType.mult,
                                           op1=mybir.AluOpType.mult)
            nc.scalar.activation(shf[:, s, 6:7], t6[:, s, None],
                                 mybir.ActivationFunctionType.Copy,
                                 bias=-0.31539156525252005,
                                 scale=0.31539156525252005)
            sh_eng_.scalar_tensor_tensor(out=shf[:, s, 7:8], in0=ux, scalar=c2,
                                           in1=uz, op0=mybir.AluOpType.mult,
                                           op1=mybir.AluOpType.mult)
            sh_eng_.scalar_tensor_tensor(out=t8[:, s, None], in0=ux,
                                           scalar=0.5 * c2, in1=ux,
                                           op0=mybir.AluOpType.mult,
                                           op1=mybir.AluOpType.mult)
            sh_eng_.scalar_tensor_tensor(out=t8b[:, s, None], in0=uy,
                                           scalar=-0.5 * c2, in1=uy,
                                           op0=mybir.AluOpType.mult,
                                           op1=mybir.AluOpType.mult)
            sh_eng_.tensor_add(out=shf[:, s, 8:9], in0=t8[:, s, None],
                                 in1=t8b[:, s, None])
            # rad = sinc(k r / C) * mask  (bf16)

        # ---- persistent Z tiles, double-buffered across superblocks ----
        zs2 = [[zper.tile([128, ZCOLS], bf16, name=f"z{d}_{i}")
                for i in range(SUPER)] for d in range(2)]
        for zset in zs2:
            for z in zset:
                zap = z[:]
                nc.vector.memset(
                    AP(zap.tensor, zap.offset + 468,
                       [list(zap.ap[0]), [FPBLK, 8], [1, FPBLK - 468]]), 0.0)
        # persistent Z^T pair tiles [128, 2, 512], double-buffered
        zts2 = [[zper.tile([128, 2, 512], bf16, name=f"zt{d}_{cp}")
                 for cp in range(NCHUNK // 2)] for d in range(2)]

        sb_count = [0]

        def emit_gather(i, j):
            g = gpool.tile([128, F * B32], bf16, tag=f"g{i}")
            nc.gpsimd.indirect_dma_start(
                out=g[:], out_offset=None, in_=a2[:],
                in_offset=bass.IndirectOffsetOnAxis(
                    ap=idx_t[:, j, 0:1], axis=0))
            nc.gpsimd.indirect_dma_start(
                out=g[:], out_offset=None, in_=a2[:],
                in_offset=bass.IndirectOffsetOnAxis(
                    ap=idx_t[:, j, 1:2], axis=0),
                compute_op=mybir.AluOpType.add)
            return g

        def do_superblock(ebs, pair_map=None, po_map=None, pregathered=None,
                          defer_consume=False, z_map=None):
            zs = zs2[sb_count[0] % 2]
            zts = zts2[sb_count[0] % 2]
            sb_count[0] += 1
            nebs = len(ebs)
            ne = nebs * 128
            for i, j in enumerate(ebs):
                if pregathered is not None and j in pregathered:
                    g = pregathered[j]
                else:
                    g = emit_gather(i, j)
                # radfold writes the rad-scaled features straight into the
                # Z a=0 slice (sh_0 const is folded into W host-side);
                # alternate Pool/DVE so radfolds don't serialize behind the
                # in-order Pool queue's gathers (and vice versa)
                zap = zs[i][:]
                gap_ = g[:]
                a0sl = AP(zap.tensor, zap.offset,
                          [list(zap.ap[0]), [FPBLK, 8], [ABLK, 2], [1, BPAD]])
                gsl = AP(gap_.tensor, gap_.offset,
                         [list(gap_.ap[0]), [2 * B32, 8], [B32, 2], [1, BPAD]])
                radsl = AP(radb[:].tensor,
                           radb[:].offset + j * F,
                           [list(radb[:].ap[0]), [2, 8], [1, 2], [0, BPAD]])
                eng = nc.gpsimd if i == 3 else nc.vector
                eng.tensor_tensor(out=a0sl, in0=gsl, in1=radsl,
                                  op=mybir.AluOpType.mult)

                # Z build: 8 tensor_scalar (bf16 4x) per eblock off the
                # a0 slice
                for a in range(1, NSH):
                    zsl = AP(zap.tensor, zap.offset + a * BPAD,
                             [list(zap.ap[0]), [FPBLK, 8], [ABLK, 2], [1, BPAD]])
                    ze = (z_map or Z_ENGINE)[i][a]
                    if ze == 'a':
                        nc.scalar.activation(zsl, a0sl,
                                             mybir.ActivationFunctionType.Copy,
                                             bias=0.0,
                                             scale=shf[:, j, a:a + 1])
                    else:
                        eng2 = nc.gpsimd if ze == 'g' else nc.vector
                        eng2.tensor_scalar(out=zsl, in0=a0sl,
                                           scalar1=shf[:, j, a:a + 1],
                                           scalar2=None,
                                           op0=mybir.AluOpType.mult)

            # transpose all chunks; copy pairs psum->sbuf
            for cp in range(NCHUNK // 2):
                pt = pst.tile([128, 2, 512], bf16, tag="pt", space="PSUM")
                for h in range(2):
                    c = 2 * cp + h
                    for i in range(nebs):
                        nc.tensor.transpose(
                            out=pt[:, h, i * 128:(i + 1) * 128],
                            in_=zs[i][:, c * 128:(c + 1) * 128],
                            identity=identb[:])
                eng = (pair_map or PAIR_ENGINE)[(cp + sb_count[0]) % 16]
                dst = zts[cp][:, :, :ne]
                src = pt[:, :, :ne]
                if eng == 'v':
                    nc.vector.tensor_copy(out=dst, in_=src)
                else:
                    nc.scalar.copy(out=dst, in_=src)

            def consume():
                _consume(ebs, zts, ne, po_map)

            if defer_consume:
                return consume
            consume()
            return None

        def _consume(ebs, zts, ne, po_map):
            # matmuls + po copy + out DMA
            e0 = ebs[0] * EBLK
            for fp in range(8):
                po = psm.tile([2 * NC_OUT, 512], f32, tag="po", space="PSUM")
                for pi, (r0, r1) in enumerate(
                        [(0, 128), (128, 256), (256, 384), (384, 468)]):
                    cp, h = divmod(4 * fp + pi, 2)
                    rhs = zts[cp][:, h, :ne] if r1 - r0 == 128 \
                        else zts[cp][0:84, h, :ne]
                    nc.tensor.matmul(out=po[:, :ne], lhsT=wt[(fp, pi)],
                                     rhs=rhs, start=(pi == 0), stop=(pi == 3))
                pos = opool.tile([2 * NC_OUT, 512], bf16, tag=f"pos{fp}")
                if (po_map or PO_ENGINE)[fp] == 'v':
                    nc.vector.tensor_copy(out=pos[:, :ne], in_=po[:, :ne])
                else:
                    nc.scalar.copy(out=pos[:, :ne], in_=po[:, :ne])
                nc.sync.dma_start(
                    out=outT[fp * 100:(fp + 1) * 100, e0:e0 + ne],
                    in_=pos[:, :ne])

        # geometry sliced: slice k covers blocks for superblocks 4k..4k+3,
        # emitted just before superblock 4(k-?) ... first slice up front,
        # later slices interleave so pipeline fill stays short
        # partial superblock (1 eblock) runs FIRST: it fills the pipeline
        # quickly and the kernel drains on a fully-pipelined superblock.
        # geometry sliced: tiny first slices, then 8-block slices emitted
        # ~2 superblocks ahead
        NSB = (NBLK - 1) // SUPER  # 12 full superblocks after the partial
        g48 = emit_gather(0, 48)             # ahead of geometry: Pool queue
        emit_geometry(48, 49)
        do_superblock([NSB * SUPER], pregathered={48: g48},
                      z_map=[['v', 'a', 'v', 'a', 'v', 'a', 'v', 'a', 'v']])
        # two wide geometry slices: a 16-block slice for the fill, then
        # the rest in one go -- fewer, wider DVE ops pay far less per-op
        # init overhead than the old 6-slice cadence
        emitted = 0
        SCHED = {0: 16, 1: 48}
        for sb in range(NSB):
            need = SCHED.get(sb, emitted)
            if need > emitted:
                emit_geometry(emitted, need)
                emitted = need
            last = sb == NSB - 1
            do_superblock(list(range(sb * SUPER, (sb + 1) * SUPER)),
                          po_map=['v'] * 8 if last else None)

    if split_waits:
        _split_multi_waits(nc)
    return nc


def _get_nc():
    if "nc" not in _NC_CACHE:
        _NC_CACHE["nc"] = _build_bass()
    return _NC_CACHE["nc"]


# ----------------------------------------------------------------------------
# Host entry point
# ----------------------------------------------------------------------------
def kernel(atomic_descriptors, tp_weights, neighbour_displacements,
           neighbour_indices):
    atomic_descriptors = np.asarray(atomic_descriptors, dtype=np.float32)
    tp_weights = np.asarray(tp_weights, dtype=np.float32)
    neighbour_displacements = np.asarray(neighbour_displacements, dtype=np.float32)
    neighbour_indices = np.asarray(neighbour_indices, dtype=np.int32)

    # atom table: (A, 1, 25, 16) -> (A, 16, 32) f-major bf16
    A = atomic_descriptors.reshape(N_ATOMS, NB, F)
    a2 = np.zeros((N_ATOMS, F, B32), dtype=BF)
    a2[:, :, :NB] = A.transpose(0, 2, 1).astype(BF)
    a2 = a2.reshape(N_ATOMS, F * B32)

    wm = _build_weight_tensor(tp_weights).astype(BF)      # [4096, 100]
    # device layout [128, 32*100]: wmat[p, q*100+m] = wm[q*128+p, m]
    wmat = np.ascontiguousarray(
        wm.reshape(32, 128, 2 * NC_OUT).transpose(1, 0, 2)).reshape(128, -1)

    in_maps = []
    for c in range(N_CORES):
        idx_full = np.zeros((EPC, 2), dtype=np.int32)
        disp_full = np.ones((EPC, 3), dtype=np.float32)
        idx_full[:SHARD] = neighbour_indices[c * SHARD:(c + 1) * SHARD]
        disp_full[:SHARD] = neighbour_displacements[c * SHARD:(c + 1) * SHARD]
        # relayout to [128, NBLK, *]: edge j*128+p -> [p, j]
        idx2 = np.ascontiguousarray(
            idx_full.reshape(NBLK, 128, 2).transpose(1, 0, 2)).reshape(128, -1)
        disp4 = np.zeros((NBLK, 128, 4), dtype=np.float32)
        disp4[:, :, :3] = disp_full.reshape(NBLK, 128, 3)
        disp2 = np.ascontiguousarray(disp4.transpose(1, 0, 2)).reshape(128, -1)
        in_maps.append({"a2": a2, "idx": idx2, "disp": disp2, "wmat": wmat})

    nc = _get_nc()
    res = run_bass_kernel_spmd(nc, in_maps, core_ids=list(range(N_CORES)))

    out = np.empty((N_EDGES, 2, NB, F), dtype=np.float32)
    for c in range(N_CORES):
        oT = np.asarray(res.results[c]["outT"]).astype(np.float32)  # [800, EPC]
        # row fp*100 + 2*cc + df -> (f=2fp+df, par=cc//25, cm=cc%25)
        o = oT[:, :SHARD].reshape(8, 50, 2, SHARD)     # [fp, cc, df, e]
        o = o.transpose(3, 1, 0, 2).reshape(SHARD, 50, 16)  # [e, cc, f]
        o = o.reshape(SHARD, 2, 25, 16)
        out[c * SHARD:(c + 1) * SHARD] = o
    return out


if __name__ == "__main__":
    rng = np.random.default_rng(0)
    inputs = {
        "atomic_descriptors": rng.standard_normal(
            (N_ATOMS, 1, NB, F)).astype(np.float32),
        "tp_weights": (rng.standard_normal((len(PATHS), F)) * 0.1).astype(np.float32),
        "neighbour_displacements": (rng.standard_normal(
            (N_EDGES, 3)) * 1.5).astype(np.float32),
        "neighbour_indices": rng.integers(0, N_ATOMS, (N_EDGES, 2)).astype(np.int32),
    }
    out = kernel(**inputs)
    print("kernel ran, out shape", out.shape)

